# revision 41
# baseline (speedup 1.0000x reference)
"""MHA (RoPE + causal softmax attention + out-proj) on 8 NeuronCores.

Sharding: DP4 x TP2. Core c: batch b = c % 4, head-group g = c // 4
(8 heads per core). Each core computes a transposed partial output
outT = (y_local @ w_o_slice^T)^T in [D, L]; host sums the two head-group
partials per batch, transposes back and divides by the operand scale.

Precision strategy:
  QKV and out-proj matmuls run as hi/lo-compensated fp8e4 DoubleRow
  (2 k-tiles per MM at 0.5 cycles/row): a = a_hi + a_lo with both parts
  e4m3 and the residual UNSCALED (operands are pre-scaled on the host --
  x by 8, w_qkv by 128, w_o by 64 -- so residuals sit in e4m3's normal
  range). Then a.b ~ a_hi.b_hi + a_hi.b_lo + a_lo.b_hi: all three
  products share one scale and accumulate in a single PSUM (the lo.lo
  term is dropped, ~1e-3 relative). 3 DoubleRow MMs per 2 k-tiles =
  0.75x the bf16 cycle count. Attention itself stays bf16 (fp8 there
  fails the 2e-2 budget; measured).

Schedule (PE executes in emission order, so overlap is explicit):
  S1: QKV chunks q03,k03,v03 (group-0 staging ready at the end)
  S2: QKV chunks q47,k47,v47 interleaved with attention heads 0-3,
      woven by PE-cost so attention's Exp (ScalarE) and softmax DVE work
      hide under the QKV DoubleRow matmul stream
  S3: attention heads 4-7 (QKV pools closed, out-proj weights loaded)
  S4: out-projection (compensated DR fp8)

Attention per head: q/k reloaded transposed via DMA xbar; scores per
k-tile pair into a 2-bank PSUM; causal handled by skipping fully-masked
k-tiles, trimming diagonal tiles, and a [128,128] triangle mask.
Softmax denominator: DVE pair-sums + one ones(=64)-matmul per pair
(no per-k-tile denominator matmuls). y emitted as fp8 hi/lo for S4.
"""

import contextlib

import numpy as np
import ml_dtypes

import concourse.bass as bass
import concourse.tile as tile
import concourse.mybir as mybir
from concourse import bacc
from concourse.bass_utils import run_bass_kernel_spmd

E4 = ml_dtypes.float8_e4m3
BF16 = ml_dtypes.bfloat16
F32 = mybir.dt.float32
BF = mybir.dt.bfloat16
FP8 = mybir.dt.float8e4
DR = mybir.MatmulPerfMode.DoubleRow

B, L, D, H, HD = 4, 2048, 2048, 16, 128
NH = 8                      # heads per core
DL = NH * HD                # 1024 local head dims
ROPE_BASE = 10000.0

XS = 8.0                    # host scale on x
WS = 128.0                  # host scale on w_qkv
OS = 64.0                   # host scale on w_o
ONES_C = 64.0               # denominator constant: y_dev = (XS*WS/ONES_C)*y
OUT_SCALE = (XS * WS / ONES_C) * OS   # 1024: host divides outT by this
ALPHA = float(HD) ** -0.5 / (XS * XS * WS * WS)

LT = L // 128               # 16 L-tiles
DT = D // 128               # 16 D(contract)-tiles
NCH = 6                     # qkv chunks of 512 comps: q03,k03,v03,q47,k47,v47
QC = L // 512               # 4 q-chunks of 512
KT = L // 128               # 16 k-tiles

A_UNIT = 2.56               # relative PE cost of one QKV output tile
B_UNIT = 1.56               # weave weight: spread 4 heads over A's tail


def _chunk_kind(c):
    # chunk order: q(heads0-3), k(0-3), v(0-3), q(4-7), k(4-7), v(4-7)
    return ("q", "k", "v")[c % 3], c // 3


def _weave(*streams):
    """Advance generators round-robin, weighted by per-unit PE cost.

    streams: (generator, unit_cost) pairs. Each next() should emit about
    unit_cost worth of PE work.
    """
    acc = [0.0] * len(streams)
    alive = [True] * len(streams)
    while any(alive):
        k = min((i for i in range(len(streams)) if alive[i]),
                key=lambda i: acc[i])
        try:
            next(streams[k][0])
            acc[k] += streams[k][1]
        except StopIteration:
            alive[k] = False


def build_program(phases="ABC", la=2):
    nc = bacc.Bacc("TRN2", target_bir_lowering=False, debug=False, num_devices=8)

    xh_d = nc.dram_tensor("xh", [128, DT, L], FP8, kind="ExternalInput").ap()
    xl_d = nc.dram_tensor("xl", [128, DT, L], FP8, kind="ExternalInput").ap()
    wh_d = nc.dram_tensor("wh", [128, DT, 3 * DL], FP8, kind="ExternalInput").ap()
    wl_d = nc.dram_tensor("wl", [128, DT, 3 * DL], FP8, kind="ExternalInput").ap()
    woh_d = nc.dram_tensor("woh", [128, NH, L], FP8, kind="ExternalInput").ap()
    wol_d = nc.dram_tensor("wol", [128, NH, L], FP8, kind="ExternalInput").ap()
    chalf = nc.dram_tensor("chalf", [L, 256], BF, kind="ExternalInput").ap()
    shalf = nc.dram_tensor("shalf", [L, 256], BF, kind="ExternalInput").ap()
    tri_d = nc.dram_tensor("tri", [128, 128], BF, kind="ExternalInput").ap()
    outT = nc.dram_tensor("outT", [D, L], BF, kind="ExternalOutput").ap()

    qrot = nc.dram_tensor("qrot", [L, DL], BF, kind="Internal").ap()
    krot = nc.dram_tensor("krot", [L, DL], BF, kind="Internal").ap()
    vnat = nc.dram_tensor("vnat", [L, DL], BF, kind="Internal").ap()

    with tile.TileContext(nc) as tc, contextlib.ExitStack() as stk:
        ex = stk.enter_context
        outer = ex(tc.tile_pool(name="outer", bufs=1))
        pb = ex(tc.tile_pool(name="pBqk", bufs=2, side="right"))
        pbm = ex(tc.tile_pool(name="pBm", bufs=1, side="right"))
        bstk = contextlib.ExitStack()
        pba = bstk.enter_context(tc.tile_pool(name="pBa", bufs=la + 2))
        pbs = bstk.enter_context(tc.tile_pool(name="pBs", bufs=2))
        pbr = bstk.enter_context(tc.tile_pool(name="pBr", bufs=2))

        yh = outer.tile([128, NH, L], FP8, name="yh", tag="yh")
        yl = outer.tile([128, NH, L], FP8, name="yl", tag="yl")
        if "B" not in phases:
            nc.vector.memset(yh, 0.0)
            nc.vector.memset(yl, 0.0)
        ones_c = outer.tile([128, 128], BF, name="ones_c", tag="oc")
        nc.vector.memset(ones_c, ONES_C)
        tri = pbm.tile([128, 128], BF, name="tri", tag="tri")
        nc.sync.dma_start(out=tri, in_=tri_d)

        # -------- Phase A stream: QKV + RoPE (one yield per L-tile) --------
        astk = contextlib.ExitStack()
        pa = astk.enter_context(tc.tile_pool(name="pA", bufs=1))
        paw = astk.enter_context(tc.tile_pool(name="pAw", bufs=2))
        pat = astk.enter_context(tc.tile_pool(name="pAt", bufs=1))
        pao = astk.enter_context(tc.tile_pool(name="pAo", bufs=3))
        psa_box = []

        # per-d-pair x and chunk-0 weight tiles, DMA-interleaved so the
        # first matmuls start after ~4us instead of ~30us
        xhp, xlp, wh0p, wl0p = [], [], [], []
        c_sb = s_sb = None
        for dp in range(DT // 2):
            dd = slice(2 * dp, 2 * dp + 2)
            th = pa.tile([128, 2, L], FP8, name=f"xh{dp}", tag=f"xh{dp}")
            nc.sync.dma_start(out=th, in_=xh_d[:, dd, :])
            xhp.append(th)
            tl = pa.tile([128, 2, L], FP8, name=f"xl{dp}", tag=f"xl{dp}")
            nc.sync.dma_start(out=tl, in_=xl_d[:, dd, :])
            xlp.append(tl)
            twh = pa.tile([128, 2, 512], FP8, name=f"wh0{dp}", tag=f"wh0{dp}")
            nc.sync.dma_start(out=twh, in_=wh_d[:, dd, 0:512])
            wh0p.append(twh)
            twl = pa.tile([128, 2, 512], FP8, name=f"wl0{dp}", tag=f"wl0{dp}")
            nc.sync.dma_start(out=twl, in_=wl_d[:, dd, 0:512])
            wl0p.append(twl)
            if dp == 2:
                c_sb = pa.tile([128, LT, 256], BF, name="c_sb", tag="c_sb")
                nc.sync.dma_start(
                    out=c_sb, in_=chalf.rearrange("(i p) g -> p i g", p=128))
                s_sb = pa.tile([128, LT, 256], BF, name="s_sb", tag="s_sb")
                nc.sync.dma_start(
                    out=s_sb, in_=shalf.rearrange("(i p) g -> p i g", p=128))

        def rope_or_v(kind, grp, i, pnat):
            if kind == "v":
                vo = pao.tile([128, 512], BF, name="vo", tag="ro")
                nc.scalar.copy(out=vo, in_=pnat)
                nc.sync.dma_start(
                    out=vnat[i * 128:(i + 1) * 128,
                             grp * 512:(grp + 1) * 512],
                    in_=vo)
            else:
                x1 = pnat[:, 0::2]
                x2 = pnat[:, 1::2]
                ct = c_sb[:, i, :]
                st = s_sb[:, i, :]
                t1 = pat.tile([128, 256], F32, name="t1", tag="t1")
                nc.vector.tensor_mul(t1, x1, ct)
                t2 = pat.tile([128, 256], F32, name="t2", tag="t2")
                nc.vector.tensor_mul(t2, x2, st)
                t3 = pat.tile([128, 256], F32, name="t3", tag="t3")
                nc.vector.tensor_mul(t3, x2, ct)
                t4 = pat.tile([128, 256], F32, name="t4", tag="t4")
                nc.vector.tensor_mul(t4, x1, st)
                ro = pao.tile([128, 512], BF, name="ro", tag="ro")
                nc.vector.tensor_sub(ro[:, 0::2], t1, t2)
                nc.vector.tensor_add(ro[:, 1::2], t3, t4)
                dst = qrot if kind == "q" else krot
                nc.sync.dma_start(
                    out=dst[i * 128:(i + 1) * 128,
                            grp * 512:(grp + 1) * 512],
                    in_=ro)

        s1stk = contextlib.ExitStack()

        def a_chunk0():
            # dp-outer over L-halves with 8 PSUM banks: matmuls consume
            # x/w d-pairs as their DMAs land instead of waiting for all 16
            kind, grp = _chunk_kind(0)
            ps0 = s1stk.enter_context(
                tc.tile_pool(name="ps0", bufs=1, space="PSUM"))
            psa_box.append(ps0)
            for half in range(2):
                pns = [ps0.tile([128, 512], F32, name=f"pn{ii}",
                                tag=f"pn{ii}") for ii in range(8)]
                for dp in range(DT // 2):
                    for ii in range(8):
                        i = half * 8 + ii
                        ic = slice(i * 128, (i + 1) * 128)
                        nc.tensor.matmul(
                            pns[ii], xhp[dp][:, :, ic], wh0p[dp],
                            start=(dp == 0), stop=False, perf_mode=DR)
                        nc.tensor.matmul(
                            pns[ii], xhp[dp][:, :, ic], wl0p[dp],
                            start=False, stop=False, perf_mode=DR)
                        nc.tensor.matmul(
                            pns[ii], xlp[dp][:, :, ic], wh0p[dp],
                            start=False, stop=(dp == DT // 2 - 1),
                            perf_mode=DR)
                for ii in range(8):
                    rope_or_v(kind, grp, half * 8 + ii, pns[ii])
                    yield

        staged_c = [False] * NCH  # chunk-c staging stores all emitted

        def a_chunks(cs):
            for c in cs:
                kind, grp = _chunk_kind(c)
                if c == 0:
                    yield from a_chunk0()
                    staged_c[0] = True
                    continue
                else:
                    wh = paw.tile([128, DT, 512], FP8, name="wh", tag="wh")
                    nc.sync.dma_start(
                        out=wh, in_=wh_d[:, :, c * 512:(c + 1) * 512])
                    wl = paw.tile([128, DT, 512], FP8, name="wl", tag="wl")
                    nc.sync.dma_start(
                        out=wl, in_=wl_d[:, :, c * 512:(c + 1) * 512])
                    whs = [wh[:, slice(2 * dp, 2 * dp + 2), :]
                           for dp in range(DT // 2)]
                    wls = [wl[:, slice(2 * dp, 2 * dp + 2), :]
                           for dp in range(DT // 2)]
                psa = psa_box[-1]
                for i in range(LT):
                    if c < 3:
                        pnat = psa.tile([128, 512], F32, name=f"pn{i % 8}",
                                        tag=f"pn{i % 8}")
                    else:
                        pnat = psa.tile([128, 512], F32, name="pnat",
                                        tag="pnat")
                    ic = slice(i * 128, (i + 1) * 128)
                    for dp in range(DT // 2):
                        nc.tensor.matmul(
                            pnat, xhp[dp][:, :, ic], whs[dp],
                            start=(dp == 0), stop=False, perf_mode=DR)
                        nc.tensor.matmul(
                            pnat, xhp[dp][:, :, ic], wls[dp],
                            start=False, stop=False, perf_mode=DR)
                        nc.tensor.matmul(
                            pnat, xlp[dp][:, :, ic], whs[dp],
                            start=False, stop=(dp == DT // 2 - 1),
                            perf_mode=DR)
                    rope_or_v(kind, grp, i, pnat)
                    yield
                staged_c[c] = True

        # -------- Phase B stream: attention (one yield per pair-group) ----
        pend = {}

        def b_load(h):
            # gated per source chunk: emit each load as soon as its staging
            # chunk is fully emitted; returns True when all three are in
            g = h // 4
            d = pend.setdefault(h, {})
            if "qt" not in d and staged_c[3 * g]:
                qt = pb.tile([128, L], BF, name="qt", tag="qt")
                nc.sync.dma_start_transpose(
                    out=qt, in_=qrot[:, h * 128:(h + 1) * 128])
                d["qt"] = qt
            if "kt" not in d and staged_c[3 * g + 1]:
                kt = pb.tile([128, L], BF, name="kt", tag="kt")
                nc.sync.dma_start_transpose(
                    out=kt, in_=krot[:, h * 128:(h + 1) * 128])
                d["kt"] = kt
            if "vt" not in d and staged_c[3 * g + 2]:
                vt = pb.tile([128, KT, 128], BF, name="vt", tag="vt")
                nc.sync.dma_start(
                    out=vt,
                    in_=vnat[:, h * 128:(h + 1) * 128].rearrange(
                        "(j p) d -> p j d", p=128))
                d["vt"] = vt
            return len(d) == 3

        def b_heads(hs, pools, tail_load=None):
            pss, psy, psd, paired = pools

            def maybe_load(h2):
                if h2 is not None and len(pend.get(h2, ())) < 3:
                    b_load(h2)

            for idx, h in enumerate(hs):
                nxt = hs[idx + 1] if idx + 1 < len(hs) else tail_load
                while not b_load(h):
                    yield           # spin: let the A stream emit staging
                d = pend.pop(h)
                qt, kt, vt = d["qt"], d["kt"], d["vt"]

                for qc in range(QC):
                    if qc >= 1:
                        maybe_load(nxt)
                    nkt = 4 * qc + 4
                    npair = 2 * qc + 2      # last 2 pairs are diagonal
                    ypsum = psy.tile([128, 512], F32, name="yp", tag="yp")
                    dpsum = psd.tile([128, 512], F32, name="dp", tag="dp")
                    qs = slice(qc * 512, (qc + 1) * 512)
                    ats = {}

                    def moff(j, qc=qc):
                        # leading masked q-columns of k-tile j's scores
                        return max(0, 128 * (j - 4 * qc))

                    def emit(pg, qc=qc, qt=qt, kt=kt, ats=ats):
                        at = pba.tile([128, 2, 512], BF, name="at", tag="at")
                        if paired:
                            scp = pss.tile([128, 2, 512], F32,
                                           name="scp", tag="scp")
                            for s in range(2):
                                j = 2 * pg + s
                                n = 512 - moff(j)
                                nc.tensor.matmul(
                                    scp[:, s, :n],
                                    kt[:, j * 128:(j + 1) * 128],
                                    qt[:, qc * 512 + 512 - n:(qc + 1) * 512],
                                    start=True, stop=True)
                            n1 = 512 - moff(2 * pg + 1)
                            if moff(2 * pg) == 0:
                                # slots contiguous in the 2-bank tile: one
                                # Exp over [0 : 512+n1]
                                flat = 512 + n1
                                nc.scalar.activation(
                                    out=at.rearrange(
                                        "p a b -> p (a b)")[:, :flat],
                                    in_=scp.rearrange(
                                        "p a b -> p (a b)")[:, :flat],
                                    func=mybir.ActivationFunctionType.Exp,
                                    scale=ALPHA)
                            else:
                                for s in range(2):
                                    n = 512 - moff(2 * pg + s)
                                    nc.scalar.activation(
                                        out=at[:, s, :n], in_=scp[:, s, :n],
                                        func=mybir.ActivationFunctionType.Exp,
                                        scale=ALPHA)
                        else:
                            for s in range(2):
                                j = 2 * pg + s
                                n = 512 - moff(j)
                                scp = pss.tile([128, 512], F32, name="scp",
                                               tag="scp")
                                nc.tensor.matmul(
                                    scp[:, :n],
                                    kt[:, j * 128:(j + 1) * 128],
                                    qt[:, qc * 512 + 512 - n:(qc + 1) * 512],
                                    start=True, stop=True)
                                nc.scalar.activation(
                                    out=at[:, s, :n], in_=scp[:, :n],
                                    func=mybir.ActivationFunctionType.Exp,
                                    scale=ALPHA)
                        for s in range(2):
                            j = 2 * pg + s
                            if moff(j) or j == 4 * qc:
                                nc.vector.tensor_mul(
                                    at[:, s, :128], at[:, s, :128], tri)
                        ats[pg] = at

                    sd = None
                    for pg in range(min(la, npair)):
                        emit(pg)
                    for pg in range(npair):
                        if pg + la < npair:
                            emit(pg + la)
                        at = ats.pop(pg)
                        for s in range(2):
                            j = 2 * pg + s
                            off = moff(j)
                            nc.tensor.matmul(
                                ypsum[:, off:], vt[:, j, :],
                                at[:, s, :512 - off],
                                start=(j == 0), stop=(j == nkt - 1))
                        if pg < 2 * qc:          # full pair
                            if pg % 2 == 0:
                                sg = pbs.tile([128, 512], BF,
                                              name="sg", tag="sg")
                                nc.vector.tensor_add(
                                    sg, at[:, 0, :], at[:, 1, :])
                            else:                # merge into quad, then MM
                                nc.vector.tensor_add(sg, sg, at[:, 0, :])
                                nc.vector.tensor_add(sg, sg, at[:, 1, :])
                                nc.tensor.matmul(
                                    dpsum, ones_c, sg,
                                    start=(pg == 1), stop=False)
                        elif pg == 2 * qc:       # diagonal pair 0
                            sd = pbs.tile([128, 512], BF, name="sd", tag="sd")
                            nc.vector.tensor_scalar_add(sd, at[:, 0, :], 0.0)
                            nc.vector.tensor_add(
                                sd[:, 128:], sd[:, 128:], at[:, 1, :384])
                        else:                    # diagonal pair 1
                            nc.vector.tensor_add(
                                sd[:, 256:], sd[:, 256:], at[:, 0, :256])
                            nc.vector.tensor_add(
                                sd[:, 384:], sd[:, 384:], at[:, 1, :128])
                            nc.tensor.matmul(
                                dpsum, ones_c, sd, start=(qc == 0), stop=True)
                        yield
                        maybe_load(nxt)
                    rbs = pbr.tile([128, 512], BF, name="rbs", tag="rbs")
                    with nc.allow_low_precision("softmax recip bf16"):
                        nc.vector.reciprocal(out=rbs, in_=dpsum)
                    yf = pbr.tile([128, 512], BF, name="yf", tag="yf")
                    nc.vector.tensor_mul(yf, ypsum, rbs)
                    nc.vector.tensor_mul(yh[:, h, qs], ypsum, rbs)
                    nc.vector.tensor_sub(yl[:, h, qs], yf, yh[:, h, qs])

        # ---------------- schedule ----------------
        do_a = "A" in phases
        do_b = "B" in phases
        do_c = "C" in phases

        if do_a:
            for _ in a_chunks([0, 1, 2]):       # S1
                pass
            s1stk.close()
            psa_box.append(astk.enter_context(
                tc.tile_pool(name="psA", bufs=2, space="PSUM")))
        if do_a and do_b:
            with tc.tile_pool(name="psS2", bufs=3, space="PSUM") as pss2, \
                 tc.tile_pool(name="psY2", bufs=2, space="PSUM") as psy2, \
                 tc.tile_pool(name="psD2", bufs=1, space="PSUM") as psd2:
                _weave((a_chunks([3, 4, 5]), A_UNIT),
                       (b_heads([0, 1, 2, 3], (pss2, psy2, psd2, False),
                                tail_load=4), B_UNIT))   # S2
        elif do_a:
            for _ in a_chunks([3, 4, 5]):
                pass
        astk.close()                            # free QKV pools / PSUM

        pcw = bstk.enter_context(tc.tile_pool(name="pCw", bufs=1))
        wohp, wolp = [], []
        if do_c:
            for p in range(NH // 2):
                dd = slice(2 * p, 2 * p + 2)
                t1 = pcw.tile([128, 2, L], FP8, name=f"woh{p}", tag=f"woh{p}")
                nc.sync.dma_start(out=t1, in_=woh_d[:, dd, :])
                wohp.append(t1)
                t2 = pcw.tile([128, 2, L], FP8, name=f"wol{p}", tag=f"wol{p}")
                nc.sync.dma_start(out=t2, in_=wol_d[:, dd, :])
                wolp.append(t2)
        pss3 = bstk.enter_context(
            tc.tile_pool(name="psS3", bufs=2, space="PSUM"))
        psy3 = bstk.enter_context(
            tc.tile_pool(name="psY3", bufs=2, space="PSUM"))
        psd3 = bstk.enter_context(
            tc.tile_pool(name="psD3", bufs=2, space="PSUM"))
        if do_b:
            rest = [4, 5, 6, 7] if do_a else list(range(NH))
            for _ in b_heads(rest, (pss3, psy3, psd3, True)):   # S3
                pass

        # ---------------- S4: out-projection (reuses B pools) ----------------
        if do_c:
                for e in range(DT):
                    ec = slice(e * 128, (e + 1) * 128)
                    for qc in range(QC):
                        qs = slice(qc * 512, (qc + 1) * 512)
                        op = pss3.tile([128, 2, 512], F32, name="op",
                                       tag="scp")[:, 0, :]
                        for p in range(NH // 2):
                            dd = slice(2 * p, 2 * p + 2)
                            nc.tensor.matmul(
                                op, wohp[p][:, :, ec], yh[:, dd, qs],
                                start=(p == 0), stop=False, perf_mode=DR)
                            nc.tensor.matmul(
                                op, wohp[p][:, :, ec], yl[:, dd, qs],
                                start=False, stop=False, perf_mode=DR)
                            nc.tensor.matmul(
                                op, wolp[p][:, :, ec], yh[:, dd, qs],
                                start=False, stop=(p == NH // 2 - 1),
                                perf_mode=DR)
                        ot = pba.tile([128, 2, 512], BF, name="ot",
                                      tag="at")[:, 0, :]
                        nc.scalar.copy(out=ot, in_=op)
                        nc.sync.dma_start(
                            out=outT[e * 128:(e + 1) * 128, qs], in_=ot)
        bstk.close()
    nc.compile()
    return nc


_NC_CACHE = None


def _get_program():
    global _NC_CACHE
    if _NC_CACHE is None:
        _NC_CACHE = build_program()
    return _NC_CACHE


def _q8(a):
    return np.clip(a, -240.0, 240.0).astype(E4)


def _hilo(a):
    hi = _q8(a)
    lo = _q8(a - hi.astype(np.float32))
    return hi, lo


def _host_inputs(x, w_qkv, w_o):
    inv = 1.0 / (ROPE_BASE ** (np.arange(0, HD, 2, dtype=np.float64) / HD))
    ang = np.arange(L, dtype=np.float64)[:, None] * inv[None, :]
    chalf = np.tile(np.cos(ang), (1, 4)).astype(BF16)          # [L, 256]
    shalf = np.tile(np.sin(ang), (1, 4)).astype(BF16)
    p = np.arange(128)[:, None]
    f = np.arange(128)[None, :]
    tri = (p <= f).astype(BF16)

    def to_pdl(a, nt):  # [nt*128, cols] -> [128, nt, cols]
        return np.ascontiguousarray(
            a.reshape(nt, 128, a.shape[1]).transpose(1, 0, 2))

    xs = {}
    for b in range(B):
        xh, xl = _hilo(XS * x[b].T)
        xs[b] = (to_pdl(xh, DT), to_pdl(xl, DT))

    in_maps = []
    for c in range(8):
        b, g = c % 4, c // 4
        qr = w_qkv[g * DL:(g + 1) * DL]
        kr = w_qkv[D + g * DL:D + (g + 1) * DL]
        vr = w_qkv[2 * D + g * DL:2 * D + (g + 1) * DL]
        wqkvT = np.concatenate(
            [qr[:512], kr[:512], vr[:512],
             qr[512:], kr[512:], vr[512:]], axis=0).T  # [D, 3DL]
        wh, wl = _hilo(WS * wqkvT)
        woT = w_o[:, g * DL:(g + 1) * DL].T            # [DL, D]
        woh, wol = _hilo(OS * woT)
        in_maps.append({
            "xh": xs[b][0], "xl": xs[b][1],
            "wh": to_pdl(wh, DT), "wl": to_pdl(wl, DT),
            "woh": to_pdl(woh, NH), "wol": to_pdl(wol, NH),
            "chalf": chalf, "shalf": shalf, "tri": tri,
        })
    return in_maps


def kernel(x, w_qkv, w_o, _trace=False):
    x = np.asarray(x, dtype=np.float32)
    w_qkv = np.asarray(w_qkv, dtype=np.float32)
    w_o = np.asarray(w_o, dtype=np.float32)
    nc = _get_program()
    in_maps = _host_inputs(x, w_qkv, w_o)
    res = run_bass_kernel_spmd(nc, in_maps, core_ids=list(range(8)),
                               trace=_trace)
    kernel.last_result = res
    parts = [r["outT"].astype(np.float32) for r in res.results]
    inv_scale = np.float32(1.0 / OUT_SCALE)
    out = np.empty((B, L, D), dtype=np.float32)
    for b in range(B):
        out[b] = (parts[b] + parts[b + 4]).T * inv_scale
    return out


# revision 44
# speedup vs baseline: 1.0012x; 1.0012x over previous
"""MHA (RoPE + causal softmax attention + out-proj) on 8 NeuronCores.

Sharding: DP4 x TP2. Core c: batch b = c % 4, head-group g = c // 4
(8 heads per core). Each core computes a transposed partial output
outT = (y_local @ w_o_slice^T)^T in [D, L]; host sums the two head-group
partials per batch, transposes back and divides by the operand scale.

Precision strategy:
  QKV and out-proj matmuls run as hi/lo-compensated fp8e4 DoubleRow
  (2 k-tiles per MM at 0.5 cycles/row): a = a_hi + a_lo with both parts
  e4m3 and the residual UNSCALED (operands are pre-scaled on the host --
  x by 8, w_qkv by 128, w_o by 64 -- so residuals sit in e4m3's normal
  range). Then a.b ~ a_hi.b_hi + a_hi.b_lo + a_lo.b_hi: all three
  products share one scale and accumulate in a single PSUM (the lo.lo
  term is dropped, ~1e-3 relative). 3 DoubleRow MMs per 2 k-tiles =
  0.75x the bf16 cycle count. Attention itself stays bf16 (fp8 there
  fails the 2e-2 budget; measured).

Schedule (PE executes in emission order, so overlap is explicit):
  S1: QKV chunks q03,k03,v03 (group-0 staging ready at the end)
  S2: QKV chunks q47,k47,v47 interleaved with attention heads 0-3,
      woven by PE-cost so attention's Exp (ScalarE) and softmax DVE work
      hide under the QKV DoubleRow matmul stream
  S3: attention heads 4-7 (QKV pools closed, out-proj weights loaded)
  S4: out-projection (compensated DR fp8)

Attention per head: q/k reloaded transposed via DMA xbar; scores per
k-tile pair into a 2-bank PSUM; causal handled by skipping fully-masked
k-tiles, trimming diagonal tiles, and a [128,128] triangle mask.
Softmax denominator: DVE pair-sums + one ones(=64)-matmul per pair
(no per-k-tile denominator matmuls). y emitted as fp8 hi/lo for S4.
"""

import contextlib

import numpy as np
import ml_dtypes

import concourse.bass as bass
import concourse.tile as tile
import concourse.mybir as mybir
from concourse import bacc
from concourse.bass_utils import run_bass_kernel_spmd

E4 = ml_dtypes.float8_e4m3
BF16 = ml_dtypes.bfloat16
F32 = mybir.dt.float32
BF = mybir.dt.bfloat16
FP8 = mybir.dt.float8e4
DR = mybir.MatmulPerfMode.DoubleRow

B, L, D, H, HD = 4, 2048, 2048, 16, 128
NH = 8                      # heads per core
DL = NH * HD                # 1024 local head dims
ROPE_BASE = 10000.0

XS = 8.0                    # host scale on x
WS = 128.0                  # host scale on w_qkv
OS = 64.0                   # host scale on w_o
ONES_C = 64.0               # denominator constant: y_dev = (XS*WS/ONES_C)*y
OUT_SCALE = (XS * WS / ONES_C) * OS   # 1024: host divides outT by this
ALPHA = float(HD) ** -0.5 / (XS * XS * WS * WS)

LT = L // 128               # 16 L-tiles
DT = D // 128               # 16 D(contract)-tiles
NCH = 6                     # qkv chunks of 512 comps: q03,k03,v03,q47,k47,v47
QC = L // 512               # 4 q-chunks of 512
KT = L // 128               # 16 k-tiles

A_UNIT = 2.56               # relative PE cost of one QKV output tile
B_UNIT = 1.56               # weave weight: spread 4 heads over A's tail


def _chunk_kind(c):
    # chunk order: q(heads0-3), k(0-3), v(0-3), q(4-7), k(4-7), v(4-7)
    return ("q", "k", "v")[c % 3], c // 3


def _weave(*streams):
    """Advance generators round-robin, weighted by per-unit PE cost.

    streams: (generator, unit_cost) pairs. Each next() should emit about
    unit_cost worth of PE work.
    """
    acc = [0.0] * len(streams)
    alive = [True] * len(streams)
    while any(alive):
        k = min((i for i in range(len(streams)) if alive[i]),
                key=lambda i: acc[i])
        try:
            next(streams[k][0])
            acc[k] += streams[k][1]
        except StopIteration:
            alive[k] = False


def build_program(phases="ABC", la=2):
    nc = bacc.Bacc("TRN2", target_bir_lowering=False, debug=False, num_devices=8)

    xh_d = nc.dram_tensor("xh", [128, DT, L], FP8, kind="ExternalInput").ap()
    xl_d = nc.dram_tensor("xl", [128, DT, L], FP8, kind="ExternalInput").ap()
    wh_d = nc.dram_tensor("wh", [128, DT, 3 * DL], FP8, kind="ExternalInput").ap()
    wl_d = nc.dram_tensor("wl", [128, DT, 3 * DL], FP8, kind="ExternalInput").ap()
    woh_d = nc.dram_tensor("woh", [128, NH, L], FP8, kind="ExternalInput").ap()
    wol_d = nc.dram_tensor("wol", [128, NH, L], FP8, kind="ExternalInput").ap()
    chalf = nc.dram_tensor("chalf", [L, 256], BF, kind="ExternalInput").ap()
    shalf = nc.dram_tensor("shalf", [L, 256], BF, kind="ExternalInput").ap()
    tri_d = nc.dram_tensor("tri", [128, 128], BF, kind="ExternalInput").ap()
    outT = nc.dram_tensor("outT", [D, L], BF, kind="ExternalOutput").ap()

    qrot = nc.dram_tensor("qrot", [L, DL], BF, kind="Internal").ap()
    krot = nc.dram_tensor("krot", [L, DL], BF, kind="Internal").ap()
    vnat = nc.dram_tensor("vnat", [L, DL], BF, kind="Internal").ap()

    with tile.TileContext(nc) as tc, contextlib.ExitStack() as stk:
        ex = stk.enter_context
        outer = ex(tc.tile_pool(name="outer", bufs=1))
        pb = ex(tc.tile_pool(name="pBqk", bufs=2, side="right"))
        pbm = ex(tc.tile_pool(name="pBm", bufs=1, side="right"))
        bstk = contextlib.ExitStack()
        pba = bstk.enter_context(tc.tile_pool(name="pBa", bufs=la + 2))
        pbs = bstk.enter_context(tc.tile_pool(name="pBs", bufs=2))
        pbr = bstk.enter_context(tc.tile_pool(name="pBr", bufs=2))

        yh = outer.tile([128, NH, L], FP8, name="yh", tag="yh")
        yl = outer.tile([128, NH, L], FP8, name="yl", tag="yl")
        if "B" not in phases:
            nc.vector.memset(yh, 0.0)
            nc.vector.memset(yl, 0.0)
        ones_c = outer.tile([128, 128], BF, name="ones_c", tag="oc")
        nc.vector.memset(ones_c, ONES_C)
        tri = pbm.tile([128, 128], BF, name="tri", tag="tri")
        nc.sync.dma_start(out=tri, in_=tri_d)

        # -------- Phase A stream: QKV + RoPE (one yield per L-tile) --------
        astk = contextlib.ExitStack()
        pa = astk.enter_context(tc.tile_pool(name="pA", bufs=1))
        paw = astk.enter_context(tc.tile_pool(name="pAw", bufs=2))
        pat = astk.enter_context(tc.tile_pool(name="pAt", bufs=1))
        pao = astk.enter_context(tc.tile_pool(name="pAo", bufs=3))
        psa_box = []

        # per-d-pair x and chunk-0 weight tiles, DMA-interleaved so the
        # first matmuls start after ~4us instead of ~30us
        xhp, xlp, wh0p, wl0p = [], [], [], []
        c_sb = s_sb = None
        for dp in range(DT // 2):
            dd = slice(2 * dp, 2 * dp + 2)
            th = pa.tile([128, 2, L], FP8, name=f"xh{dp}", tag=f"xh{dp}")
            nc.sync.dma_start(out=th, in_=xh_d[:, dd, :])
            xhp.append(th)
            tl = pa.tile([128, 2, L], FP8, name=f"xl{dp}", tag=f"xl{dp}")
            nc.sync.dma_start(out=tl, in_=xl_d[:, dd, :])
            xlp.append(tl)
            twh = pa.tile([128, 2, 512], FP8, name=f"wh0{dp}", tag=f"wh0{dp}")
            nc.sync.dma_start(out=twh, in_=wh_d[:, dd, 0:512])
            wh0p.append(twh)
            twl = pa.tile([128, 2, 512], FP8, name=f"wl0{dp}", tag=f"wl0{dp}")
            nc.sync.dma_start(out=twl, in_=wl_d[:, dd, 0:512])
            wl0p.append(twl)
            if dp == 2:
                c_sb = pa.tile([128, LT, 256], BF, name="c_sb", tag="c_sb")
                nc.sync.dma_start(
                    out=c_sb, in_=chalf.rearrange("(i p) g -> p i g", p=128))
                s_sb = pa.tile([128, LT, 256], BF, name="s_sb", tag="s_sb")
                nc.sync.dma_start(
                    out=s_sb, in_=shalf.rearrange("(i p) g -> p i g", p=128))

        def rope_or_v(kind, grp, i, pnat):
            if kind == "v":
                vo = pao.tile([128, 512], BF, name="vo", tag="ro")
                nc.scalar.copy(out=vo, in_=pnat)
                nc.sync.dma_start(
                    out=vnat[i * 128:(i + 1) * 128,
                             grp * 512:(grp + 1) * 512],
                    in_=vo)
            else:
                x1 = pnat[:, 0::2]
                x2 = pnat[:, 1::2]
                ct = c_sb[:, i, :]
                st = s_sb[:, i, :]
                t1 = pat.tile([128, 256], F32, name="t1", tag="t1")
                nc.vector.tensor_mul(t1, x1, ct)
                t2 = pat.tile([128, 256], F32, name="t2", tag="t2")
                nc.vector.tensor_mul(t2, x2, st)
                t3 = pat.tile([128, 256], F32, name="t3", tag="t3")
                nc.vector.tensor_mul(t3, x2, ct)
                t4 = pat.tile([128, 256], F32, name="t4", tag="t4")
                nc.vector.tensor_mul(t4, x1, st)
                ro = pao.tile([128, 512], BF, name="ro", tag="ro")
                nc.vector.tensor_sub(ro[:, 0::2], t1, t2)
                nc.vector.tensor_add(ro[:, 1::2], t3, t4)
                dst = qrot if kind == "q" else krot
                nc.sync.dma_start(
                    out=dst[i * 128:(i + 1) * 128,
                            grp * 512:(grp + 1) * 512],
                    in_=ro)

        s1stk = contextlib.ExitStack()

        def a_chunk0():
            # dp-outer over L-halves with 8 PSUM banks: matmuls consume
            # x/w d-pairs as their DMAs land instead of waiting for all 16
            kind, grp = _chunk_kind(0)
            ps0 = s1stk.enter_context(
                tc.tile_pool(name="ps0", bufs=1, space="PSUM"))
            psa_box.append(ps0)
            for half in range(2):
                pns = [ps0.tile([128, 512], F32, name=f"pn{ii}",
                                tag=f"pn{ii}") for ii in range(8)]
                for dp in range(DT // 2):
                    for ii in range(8):
                        i = half * 8 + ii
                        ic = slice(i * 128, (i + 1) * 128)
                        nc.tensor.matmul(
                            pns[ii], xhp[dp][:, :, ic], wh0p[dp],
                            start=(dp == 0), stop=False, perf_mode=DR)
                        nc.tensor.matmul(
                            pns[ii], xhp[dp][:, :, ic], wl0p[dp],
                            start=False, stop=False, perf_mode=DR)
                        nc.tensor.matmul(
                            pns[ii], xlp[dp][:, :, ic], wh0p[dp],
                            start=False, stop=(dp == DT // 2 - 1),
                            perf_mode=DR)
                for ii in range(8):
                    rope_or_v(kind, grp, half * 8 + ii, pns[ii])
                    yield

        staged_c = [False] * NCH  # chunk-c staging stores all emitted

        def a_chunks(cs):
            for c in cs:
                kind, grp = _chunk_kind(c)
                if c == 0:
                    yield from a_chunk0()
                    staged_c[0] = True
                    continue
                else:
                    wh = paw.tile([128, DT, 512], FP8, name="wh", tag="wh")
                    nc.sync.dma_start(
                        out=wh, in_=wh_d[:, :, c * 512:(c + 1) * 512])
                    wl = paw.tile([128, DT, 512], FP8, name="wl", tag="wl")
                    nc.sync.dma_start(
                        out=wl, in_=wl_d[:, :, c * 512:(c + 1) * 512])
                    whs = [wh[:, slice(2 * dp, 2 * dp + 2), :]
                           for dp in range(DT // 2)]
                    wls = [wl[:, slice(2 * dp, 2 * dp + 2), :]
                           for dp in range(DT // 2)]
                psa = psa_box[-1]
                for i in range(LT):
                    if c < 3:
                        pnat = psa.tile([128, 512], F32, name=f"pn{i % 8}",
                                        tag=f"pn{i % 8}")
                    else:
                        pnat = psa.tile([128, 512], F32, name="pnat",
                                        tag="pnat")
                    ic = slice(i * 128, (i + 1) * 128)
                    for dp in range(DT // 2):
                        nc.tensor.matmul(
                            pnat, xhp[dp][:, :, ic], whs[dp],
                            start=(dp == 0), stop=False, perf_mode=DR)
                        nc.tensor.matmul(
                            pnat, xhp[dp][:, :, ic], wls[dp],
                            start=False, stop=False, perf_mode=DR)
                        nc.tensor.matmul(
                            pnat, xlp[dp][:, :, ic], whs[dp],
                            start=False, stop=(dp == DT // 2 - 1),
                            perf_mode=DR)
                    rope_or_v(kind, grp, i, pnat)
                    yield
                staged_c[c] = True

        # -------- Phase B stream: attention (one yield per pair-group) ----
        pend = {}

        def b_load(h):
            # gated per source chunk: emit each load as soon as its staging
            # chunk is fully emitted; returns True when all three are in
            g = h // 4
            d = pend.setdefault(h, {})
            if "qt" not in d and staged_c[3 * g]:
                qt = pb.tile([128, L], BF, name="qt", tag="qt")
                nc.sync.dma_start_transpose(
                    out=qt, in_=qrot[:, h * 128:(h + 1) * 128])
                d["qt"] = qt
            if "kt" not in d and staged_c[3 * g + 1]:
                kt = pb.tile([128, L], BF, name="kt", tag="kt")
                nc.sync.dma_start_transpose(
                    out=kt, in_=krot[:, h * 128:(h + 1) * 128])
                d["kt"] = kt
            if "vt" not in d and staged_c[3 * g + 2]:
                vt = pb.tile([128, KT, 128], BF, name="vt", tag="vt")
                nc.sync.dma_start(
                    out=vt,
                    in_=vnat[:, h * 128:(h + 1) * 128].rearrange(
                        "(j p) d -> p j d", p=128))
                d["vt"] = vt
            return len(d) == 3

        def b_heads(hs, pools, tail_load=None):
            pss, psy, psd, paired = pools

            def maybe_load(h2):
                if h2 is not None and len(pend.get(h2, ())) < 3:
                    b_load(h2)

            for idx, h in enumerate(hs):
                nxt = hs[idx + 1] if idx + 1 < len(hs) else tail_load
                while not b_load(h):
                    yield           # spin: let the A stream emit staging
                d = pend.pop(h)
                qt, kt, vt = d["qt"], d["kt"], d["vt"]

                for qn, qc in enumerate(reversed(range(QC))):
                    if qn >= 1:
                        maybe_load(nxt)
                    nkt = 4 * qc + 4
                    npair = 2 * qc + 2      # last 2 pairs are diagonal
                    ypsum = psy.tile([128, 512], F32, name="yp", tag="yp")
                    dpsum = psd.tile([128, 512], F32, name="dp", tag="dp")
                    qs = slice(qc * 512, (qc + 1) * 512)
                    ats = {}

                    def moff(j, qc=qc):
                        # leading masked q-columns of k-tile j's scores
                        return max(0, 128 * (j - 4 * qc))

                    def emit(pg, qc=qc, qt=qt, kt=kt, ats=ats):
                        at = pba.tile([128, 2, 512], BF, name="at", tag="at")
                        if paired:
                            scp = pss.tile([128, 2, 512], F32,
                                           name="scp", tag="scp")
                            for s in range(2):
                                j = 2 * pg + s
                                n = 512 - moff(j)
                                nc.tensor.matmul(
                                    scp[:, s, :n],
                                    kt[:, j * 128:(j + 1) * 128],
                                    qt[:, qc * 512 + 512 - n:(qc + 1) * 512],
                                    start=True, stop=True)
                            n1 = 512 - moff(2 * pg + 1)
                            if moff(2 * pg) == 0:
                                # slots contiguous in the 2-bank tile: one
                                # Exp over [0 : 512+n1]
                                flat = 512 + n1
                                nc.scalar.activation(
                                    out=at.rearrange(
                                        "p a b -> p (a b)")[:, :flat],
                                    in_=scp.rearrange(
                                        "p a b -> p (a b)")[:, :flat],
                                    func=mybir.ActivationFunctionType.Exp,
                                    scale=ALPHA)
                            else:
                                for s in range(2):
                                    n = 512 - moff(2 * pg + s)
                                    nc.scalar.activation(
                                        out=at[:, s, :n], in_=scp[:, s, :n],
                                        func=mybir.ActivationFunctionType.Exp,
                                        scale=ALPHA)
                        else:
                            for s in range(2):
                                j = 2 * pg + s
                                n = 512 - moff(j)
                                scp = pss.tile([128, 512], F32, name="scp",
                                               tag="scp")
                                nc.tensor.matmul(
                                    scp[:, :n],
                                    kt[:, j * 128:(j + 1) * 128],
                                    qt[:, qc * 512 + 512 - n:(qc + 1) * 512],
                                    start=True, stop=True)
                                nc.scalar.activation(
                                    out=at[:, s, :n], in_=scp[:, :n],
                                    func=mybir.ActivationFunctionType.Exp,
                                    scale=ALPHA)
                        for s in range(2):
                            j = 2 * pg + s
                            if moff(j) or j == 4 * qc:
                                nc.vector.tensor_mul(
                                    at[:, s, :128], at[:, s, :128], tri)
                        ats[pg] = at

                    sd = None
                    for pg in range(min(la, npair)):
                        emit(pg)
                    for pg in range(npair):
                        if pg + la < npair:
                            emit(pg + la)
                        at = ats.pop(pg)
                        for s in range(2):
                            j = 2 * pg + s
                            off = moff(j)
                            nc.tensor.matmul(
                                ypsum[:, off:], vt[:, j, :],
                                at[:, s, :512 - off],
                                start=(j == 0), stop=(j == nkt - 1))
                        if pg < 2 * qc:          # full pair
                            if pg % 2 == 0:
                                sg = pbs.tile([128, 512], BF,
                                              name="sg", tag="sg")
                                nc.vector.tensor_add(
                                    sg, at[:, 0, :], at[:, 1, :])
                            else:                # merge into quad, then MM
                                nc.vector.tensor_add(sg, sg, at[:, 0, :])
                                nc.vector.tensor_add(sg, sg, at[:, 1, :])
                                nc.tensor.matmul(
                                    dpsum, ones_c, sg,
                                    start=(pg == 1), stop=False)
                        elif pg == 2 * qc:       # diagonal pair 0
                            sd = pbs.tile([128, 512], BF, name="sd", tag="sd")
                            nc.vector.tensor_scalar_add(sd, at[:, 0, :], 0.0)
                            nc.vector.tensor_add(
                                sd[:, 128:], sd[:, 128:], at[:, 1, :384])
                        else:                    # diagonal pair 1
                            nc.vector.tensor_add(
                                sd[:, 256:], sd[:, 256:], at[:, 0, :256])
                            nc.vector.tensor_add(
                                sd[:, 384:], sd[:, 384:], at[:, 1, :128])
                            nc.tensor.matmul(
                                dpsum, ones_c, sd, start=(qc == 0), stop=True)
                        yield
                        maybe_load(nxt)
                    rbs = pbr.tile([128, 512], BF, name="rbs", tag="rbs")
                    with nc.allow_low_precision("softmax recip bf16"):
                        nc.vector.reciprocal(out=rbs, in_=dpsum)
                    yf = pbr.tile([128, 512], BF, name="yf", tag="yf")
                    nc.vector.tensor_mul(yf, ypsum, rbs)
                    nc.vector.tensor_mul(yh[:, h, qs], ypsum, rbs)
                    nc.vector.tensor_sub(yl[:, h, qs], yf, yh[:, h, qs])

        # ---------------- schedule ----------------
        do_a = "A" in phases
        do_b = "B" in phases
        do_c = "C" in phases

        if do_a:
            for _ in a_chunks([0, 1, 2]):       # S1
                pass
            s1stk.close()
            psa_box.append(astk.enter_context(
                tc.tile_pool(name="psA", bufs=2, space="PSUM")))
        if do_a and do_b:
            with tc.tile_pool(name="psS2", bufs=3, space="PSUM") as pss2, \
                 tc.tile_pool(name="psY2", bufs=2, space="PSUM") as psy2, \
                 tc.tile_pool(name="psD2", bufs=1, space="PSUM") as psd2:
                _weave((a_chunks([3, 4, 5]), A_UNIT),
                       (b_heads([0, 1, 2, 3], (pss2, psy2, psd2, False),
                                tail_load=4), B_UNIT))   # S2
        elif do_a:
            for _ in a_chunks([3, 4, 5]):
                pass
        astk.close()                            # free QKV pools / PSUM

        pcw = bstk.enter_context(tc.tile_pool(name="pCw", bufs=1))
        wohp, wolp = [], []
        if do_c:
            for p in range(NH // 2):
                dd = slice(2 * p, 2 * p + 2)
                t1 = pcw.tile([128, 2, L], FP8, name=f"woh{p}", tag=f"woh{p}")
                nc.sync.dma_start(out=t1, in_=woh_d[:, dd, :])
                wohp.append(t1)
                t2 = pcw.tile([128, 2, L], FP8, name=f"wol{p}", tag=f"wol{p}")
                nc.sync.dma_start(out=t2, in_=wol_d[:, dd, :])
                wolp.append(t2)
        pss3 = bstk.enter_context(
            tc.tile_pool(name="psS3", bufs=2, space="PSUM"))
        psy3 = bstk.enter_context(
            tc.tile_pool(name="psY3", bufs=2, space="PSUM"))
        psd3 = bstk.enter_context(
            tc.tile_pool(name="psD3", bufs=2, space="PSUM"))
        if do_b:
            rest = [4, 5, 6, 7] if do_a else list(range(NH))
            for _ in b_heads(rest, (pss3, psy3, psd3, True)):   # S3
                pass

        # ---------------- S4: out-projection (reuses B pools) ----------------
        if do_c:
                for e in range(DT):
                    ec = slice(e * 128, (e + 1) * 128)
                    for qc in range(QC):
                        qs = slice(qc * 512, (qc + 1) * 512)
                        op = pss3.tile([128, 2, 512], F32, name="op",
                                       tag="scp")[:, 0, :]
                        for p in range(NH // 2):
                            dd = slice(2 * p, 2 * p + 2)
                            nc.tensor.matmul(
                                op, wohp[p][:, :, ec], yh[:, dd, qs],
                                start=(p == 0), stop=False, perf_mode=DR)
                            nc.tensor.matmul(
                                op, wohp[p][:, :, ec], yl[:, dd, qs],
                                start=False, stop=False, perf_mode=DR)
                            nc.tensor.matmul(
                                op, wolp[p][:, :, ec], yh[:, dd, qs],
                                start=False, stop=(p == NH // 2 - 1),
                                perf_mode=DR)
                        ot = pba.tile([128, 2, 512], BF, name="ot",
                                      tag="at")[:, 0, :]
                        nc.scalar.copy(out=ot, in_=op)
                        nc.sync.dma_start(
                            out=outT[e * 128:(e + 1) * 128, qs], in_=ot)
        bstk.close()
    nc.compile()
    return nc


_NC_CACHE = None


def _get_program():
    global _NC_CACHE
    if _NC_CACHE is None:
        _NC_CACHE = build_program()
    return _NC_CACHE


def _q8(a):
    return np.clip(a, -240.0, 240.0).astype(E4)


def _hilo(a):
    hi = _q8(a)
    lo = _q8(a - hi.astype(np.float32))
    return hi, lo


def _host_inputs(x, w_qkv, w_o):
    inv = 1.0 / (ROPE_BASE ** (np.arange(0, HD, 2, dtype=np.float64) / HD))
    ang = np.arange(L, dtype=np.float64)[:, None] * inv[None, :]
    chalf = np.tile(np.cos(ang), (1, 4)).astype(BF16)          # [L, 256]
    shalf = np.tile(np.sin(ang), (1, 4)).astype(BF16)
    p = np.arange(128)[:, None]
    f = np.arange(128)[None, :]
    tri = (p <= f).astype(BF16)

    def to_pdl(a, nt):  # [nt*128, cols] -> [128, nt, cols]
        return np.ascontiguousarray(
            a.reshape(nt, 128, a.shape[1]).transpose(1, 0, 2))

    xs = {}
    for b in range(B):
        xh, xl = _hilo(XS * x[b].T)
        xs[b] = (to_pdl(xh, DT), to_pdl(xl, DT))

    in_maps = []
    for c in range(8):
        b, g = c % 4, c // 4
        qr = w_qkv[g * DL:(g + 1) * DL]
        kr = w_qkv[D + g * DL:D + (g + 1) * DL]
        vr = w_qkv[2 * D + g * DL:2 * D + (g + 1) * DL]
        wqkvT = np.concatenate(
            [qr[:512], kr[:512], vr[:512],
             qr[512:], kr[512:], vr[512:]], axis=0).T  # [D, 3DL]
        wh, wl = _hilo(WS * wqkvT)
        woT = w_o[:, g * DL:(g + 1) * DL].T            # [DL, D]
        woh, wol = _hilo(OS * woT)
        in_maps.append({
            "xh": xs[b][0], "xl": xs[b][1],
            "wh": to_pdl(wh, DT), "wl": to_pdl(wl, DT),
            "woh": to_pdl(woh, NH), "wol": to_pdl(wol, NH),
            "chalf": chalf, "shalf": shalf, "tri": tri,
        })
    return in_maps


def kernel(x, w_qkv, w_o, _trace=False):
    x = np.asarray(x, dtype=np.float32)
    w_qkv = np.asarray(w_qkv, dtype=np.float32)
    w_o = np.asarray(w_o, dtype=np.float32)
    nc = _get_program()
    in_maps = _host_inputs(x, w_qkv, w_o)
    res = run_bass_kernel_spmd(nc, in_maps, core_ids=list(range(8)),
                               trace=_trace)
    kernel.last_result = res
    parts = [r["outT"].astype(np.float32) for r in res.results]
    inv_scale = np.float32(1.0 / OUT_SCALE)
    out = np.empty((B, L, D), dtype=np.float32)
    for b in range(B):
        out[b] = (parts[b] + parts[b + 4]).T * inv_scale
    return out


# revision 50
# speedup vs baseline: 1.0034x; 1.0022x over previous
"""MHA (RoPE + causal softmax attention + out-proj) on 8 NeuronCores.

Sharding: DP4 x TP2. Core c: batch b = c % 4, head-group g = c // 4
(8 heads per core). Each core computes a transposed partial output
outT = (y_local @ w_o_slice^T)^T in [D, L]; host sums the two head-group
partials per batch, transposes back and divides by the operand scale.

Precision strategy:
  QKV and out-proj matmuls run as hi/lo-compensated fp8e4 DoubleRow
  (2 k-tiles per MM at 0.5 cycles/row): a = a_hi + a_lo with both parts
  e4m3 and the residual UNSCALED (operands are pre-scaled on the host --
  x by 8, w_qkv by 128, w_o by 64 -- so residuals sit in e4m3's normal
  range). Then a.b ~ a_hi.b_hi + a_hi.b_lo + a_lo.b_hi: all three
  products share one scale and accumulate in a single PSUM (the lo.lo
  term is dropped, ~1e-3 relative). 3 DoubleRow MMs per 2 k-tiles =
  0.75x the bf16 cycle count. Attention itself stays bf16 (fp8 there
  fails the 2e-2 budget; measured).

Schedule (PE executes in emission order, so overlap is explicit):
  S1: QKV chunks q03,k03,v03 (group-0 staging ready at the end)
  S2: QKV chunks q47,k47,v47 interleaved with attention heads 0-3,
      woven by PE-cost so attention's Exp (ScalarE) and softmax DVE work
      hide under the QKV DoubleRow matmul stream
  S3: attention heads 4-7 (QKV pools closed, out-proj weights loaded)
  S4: out-projection (compensated DR fp8)

Attention per head: q/k reloaded transposed via DMA xbar; scores per
k-tile pair into a 2-bank PSUM; causal handled by skipping fully-masked
k-tiles, trimming diagonal tiles, and a [128,128] triangle mask.
Softmax denominator: DVE pair-sums + one ones(=64)-matmul per pair
(no per-k-tile denominator matmuls). y emitted as fp8 hi/lo for S4.
"""

import contextlib

import numpy as np
import ml_dtypes

import concourse.bass as bass
import concourse.tile as tile
import concourse.mybir as mybir
from concourse import bacc
from concourse.bass_utils import run_bass_kernel_spmd

E4 = ml_dtypes.float8_e4m3
BF16 = ml_dtypes.bfloat16
F32 = mybir.dt.float32
BF = mybir.dt.bfloat16
FP8 = mybir.dt.float8e4
DR = mybir.MatmulPerfMode.DoubleRow

B, L, D, H, HD = 4, 2048, 2048, 16, 128
NH = 8                      # heads per core
DL = NH * HD                # 1024 local head dims
ROPE_BASE = 10000.0

XS = 8.0                    # host scale on x
WS = 128.0                  # host scale on w_qkv
OS = 64.0                   # host scale on w_o
ONES_C = 64.0               # denominator constant: y_dev = (XS*WS/ONES_C)*y
OUT_SCALE = (XS * WS / ONES_C) * OS   # 1024: host divides outT by this
ALPHA = float(HD) ** -0.5 / (XS * XS * WS * WS)

LT = L // 128               # 16 L-tiles
DT = D // 128               # 16 D(contract)-tiles
NCH = 6                     # qkv chunks of 512 comps: q03,k03,v03,q47,k47,v47
QC = L // 512               # 4 q-chunks of 512
KT = L // 128               # 16 k-tiles

A_UNIT = 2.56               # relative PE cost of one QKV output tile
B_UNIT = 1.56               # weave weight: spread 4 heads over A's tail


def _chunk_kind(c):
    # chunk order: q(heads0-3), k(0-3), v(0-3), q(4-7), k(4-7), v(4-7)
    return ("q", "k", "v")[c % 3], c // 3


def _weave(*streams):
    """Advance generators round-robin, weighted by per-unit PE cost.

    streams: (generator, unit_cost) pairs. Each next() should emit about
    unit_cost worth of PE work.
    """
    acc = [0.0] * len(streams)
    alive = [True] * len(streams)
    while any(alive):
        k = min((i for i in range(len(streams)) if alive[i]),
                key=lambda i: acc[i])
        try:
            next(streams[k][0])
            acc[k] += streams[k][1]
        except StopIteration:
            alive[k] = False


def build_program(phases="ABC", la=2):
    nc = bacc.Bacc("TRN2", target_bir_lowering=False, debug=False, num_devices=8)

    xh_d = nc.dram_tensor("xh", [128, DT, L], FP8, kind="ExternalInput").ap()
    xl_d = nc.dram_tensor("xl", [128, DT, L], FP8, kind="ExternalInput").ap()
    wh_d = nc.dram_tensor("wh", [128, DT, 3 * DL], FP8, kind="ExternalInput").ap()
    wl_d = nc.dram_tensor("wl", [128, DT, 3 * DL], FP8, kind="ExternalInput").ap()
    woh_d = nc.dram_tensor("woh", [128, NH, L], FP8, kind="ExternalInput").ap()
    wol_d = nc.dram_tensor("wol", [128, NH, L], FP8, kind="ExternalInput").ap()
    chalf = nc.dram_tensor("chalf", [L, 256], BF, kind="ExternalInput").ap()
    shalf = nc.dram_tensor("shalf", [L, 256], BF, kind="ExternalInput").ap()
    tri_d = nc.dram_tensor("tri", [128, 128], BF, kind="ExternalInput").ap()
    outT = nc.dram_tensor("outT", [D, L], BF, kind="ExternalOutput").ap()

    qrot = nc.dram_tensor("qrot", [L, DL], BF, kind="Internal").ap()
    krot = nc.dram_tensor("krot", [L, DL], BF, kind="Internal").ap()
    vnat = nc.dram_tensor("vnat", [L, DL], BF, kind="Internal").ap()

    with tile.TileContext(nc) as tc, contextlib.ExitStack() as stk:
        ex = stk.enter_context
        outer = ex(tc.tile_pool(name="outer", bufs=1))
        pb = ex(tc.tile_pool(name="pBqk", bufs=2, side="right"))
        pbm = ex(tc.tile_pool(name="pBm", bufs=1, side="right"))
        bstk = contextlib.ExitStack()
        pba = bstk.enter_context(tc.tile_pool(name="pBa", bufs=la + 2))
        pbs = bstk.enter_context(tc.tile_pool(name="pBs", bufs=2))
        pbr = bstk.enter_context(tc.tile_pool(name="pBr", bufs=2))

        yh = outer.tile([128, NH, L], FP8, name="yh", tag="yh")
        yl = outer.tile([128, NH, L], FP8, name="yl", tag="yl")
        if "B" not in phases:
            nc.vector.memset(yh, 0.0)
            nc.vector.memset(yl, 0.0)
        ones_c = outer.tile([128, 128], BF, name="ones_c", tag="oc")
        nc.vector.memset(ones_c, ONES_C)
        tri = pbm.tile([128, 128], BF, name="tri", tag="tri")
        nc.sync.dma_start(out=tri, in_=tri_d)

        # -------- Phase A stream: QKV + RoPE (one yield per L-tile) --------
        astk = contextlib.ExitStack()
        pa = astk.enter_context(tc.tile_pool(name="pA", bufs=1))
        paw = astk.enter_context(tc.tile_pool(name="pAw", bufs=2))
        pat = astk.enter_context(tc.tile_pool(name="pAt", bufs=1))
        pao = astk.enter_context(tc.tile_pool(name="pAo", bufs=3))
        psa_box = []

        # per-d-pair x and chunk-0 weight tiles, DMA-interleaved so the
        # first matmuls start after ~4us instead of ~30us
        xhp, xlp, wh0p, wl0p = [], [], [], []
        c_sb = s_sb = None
        for dp in range(DT // 2):
            dd = slice(2 * dp, 2 * dp + 2)
            th = pa.tile([128, 2, L], FP8, name=f"xh{dp}", tag=f"xh{dp}")
            nc.sync.dma_start(out=th, in_=xh_d[:, dd, :])
            xhp.append(th)
            tl = pa.tile([128, 2, L], FP8, name=f"xl{dp}", tag=f"xl{dp}")
            nc.sync.dma_start(out=tl, in_=xl_d[:, dd, :])
            xlp.append(tl)
            twh = pa.tile([128, 2, 512], FP8, name=f"wh0{dp}", tag=f"wh0{dp}")
            nc.sync.dma_start(out=twh, in_=wh_d[:, dd, 0:512])
            wh0p.append(twh)
            twl = pa.tile([128, 2, 512], FP8, name=f"wl0{dp}", tag=f"wl0{dp}")
            nc.sync.dma_start(out=twl, in_=wl_d[:, dd, 0:512])
            wl0p.append(twl)
            if dp == 2:
                c_sb = pa.tile([128, LT, 256], BF, name="c_sb", tag="c_sb")
                nc.sync.dma_start(
                    out=c_sb, in_=chalf.rearrange("(i p) g -> p i g", p=128))
                s_sb = pa.tile([128, LT, 256], BF, name="s_sb", tag="s_sb")
                nc.sync.dma_start(
                    out=s_sb, in_=shalf.rearrange("(i p) g -> p i g", p=128))

        def rope_or_v(kind, grp, i, pnat):
            if kind == "v":
                vo = pao.tile([128, 512], BF, name="vo", tag="ro")
                nc.scalar.copy(out=vo, in_=pnat)
                nc.sync.dma_start(
                    out=vnat[i * 128:(i + 1) * 128,
                             grp * 512:(grp + 1) * 512],
                    in_=vo)
            else:
                x1 = pnat[:, 0::2]
                x2 = pnat[:, 1::2]
                ct = c_sb[:, i, :]
                st = s_sb[:, i, :]
                t1 = pat.tile([128, 256], F32, name="t1", tag="t1")
                nc.vector.tensor_mul(t1, x1, ct)
                t2 = pat.tile([128, 256], F32, name="t2", tag="t2")
                nc.vector.tensor_mul(t2, x2, st)
                t3 = pat.tile([128, 256], F32, name="t3", tag="t3")
                nc.vector.tensor_mul(t3, x2, ct)
                t4 = pat.tile([128, 256], F32, name="t4", tag="t4")
                nc.vector.tensor_mul(t4, x1, st)
                ro = pao.tile([128, 512], BF, name="ro", tag="ro")
                nc.vector.tensor_sub(ro[:, 0::2], t1, t2)
                nc.vector.tensor_add(ro[:, 1::2], t3, t4)
                dst = qrot if kind == "q" else krot
                nc.sync.dma_start(
                    out=dst[i * 128:(i + 1) * 128,
                            grp * 512:(grp + 1) * 512],
                    in_=ro)

        s1stk = contextlib.ExitStack()

        def a_chunk0():
            # dp-outer over L-halves with 8 PSUM banks: matmuls consume
            # x/w d-pairs as their DMAs land instead of waiting for all 16
            kind, grp = _chunk_kind(0)
            ps0 = s1stk.enter_context(
                tc.tile_pool(name="ps0", bufs=1, space="PSUM"))
            psa_box.append(ps0)
            for half in range(2):
                pns = [ps0.tile([128, 512], F32, name=f"pn{ii}",
                                tag=f"pn{ii}") for ii in range(8)]
                for dp in range(DT // 2):
                    for ii in range(8):
                        i = half * 8 + ii
                        ic = slice(i * 128, (i + 1) * 128)
                        nc.tensor.matmul(
                            pns[ii], xhp[dp][:, :, ic], wh0p[dp],
                            start=(dp == 0), stop=False, perf_mode=DR)
                        nc.tensor.matmul(
                            pns[ii], xhp[dp][:, :, ic], wl0p[dp],
                            start=False, stop=False, perf_mode=DR)
                        nc.tensor.matmul(
                            pns[ii], xlp[dp][:, :, ic], wh0p[dp],
                            start=False, stop=(dp == DT // 2 - 1),
                            perf_mode=DR)
                for ii in range(8):
                    rope_or_v(kind, grp, half * 8 + ii, pns[ii])
                    yield

        staged_c = [False] * NCH  # chunk-c staging stores all emitted

        def a_chunks(cs):
            for c in cs:
                kind, grp = _chunk_kind(c)
                if c == 0:
                    yield from a_chunk0()
                    staged_c[0] = True
                    continue
                else:
                    wh = paw.tile([128, DT, 512], FP8, name="wh", tag="wh")
                    nc.sync.dma_start(
                        out=wh, in_=wh_d[:, :, c * 512:(c + 1) * 512])
                    wl = paw.tile([128, DT, 512], FP8, name="wl", tag="wl")
                    nc.sync.dma_start(
                        out=wl, in_=wl_d[:, :, c * 512:(c + 1) * 512])
                    whs = [wh[:, slice(2 * dp, 2 * dp + 2), :]
                           for dp in range(DT // 2)]
                    wls = [wl[:, slice(2 * dp, 2 * dp + 2), :]
                           for dp in range(DT // 2)]
                psa = psa_box[-1]
                for i in range(LT):
                    if c < 3:
                        pnat = psa.tile([128, 512], F32, name=f"pn{i % 8}",
                                        tag=f"pn{i % 8}")
                    else:
                        pnat = psa.tile([128, 512], F32, name="pnat",
                                        tag="pnat")
                    ic = slice(i * 128, (i + 1) * 128)
                    for dp in range(DT // 2):
                        nc.tensor.matmul(
                            pnat, xhp[dp][:, :, ic], whs[dp],
                            start=(dp == 0), stop=False, perf_mode=DR)
                        nc.tensor.matmul(
                            pnat, xhp[dp][:, :, ic], wls[dp],
                            start=False, stop=False, perf_mode=DR)
                        nc.tensor.matmul(
                            pnat, xlp[dp][:, :, ic], whs[dp],
                            start=False, stop=(dp == DT // 2 - 1),
                            perf_mode=DR)
                    rope_or_v(kind, grp, i, pnat)
                    yield
                staged_c[c] = True

        # -------- Phase B stream: attention (one yield per pair-group) ----
        pend = {}

        def b_load(h):
            # gated per source chunk: emit each load as soon as its staging
            # chunk is fully emitted; returns True when all three are in
            g = h // 4
            d = pend.setdefault(h, {})
            if "qt" not in d and staged_c[3 * g]:
                qt = pb.tile([128, L], BF, name="qt", tag="qt")
                nc.sync.dma_start_transpose(
                    out=qt, in_=qrot[:, h * 128:(h + 1) * 128])
                d["qt"] = qt
            if "kt" not in d and staged_c[3 * g + 1]:
                kt = pb.tile([128, L], BF, name="kt", tag="kt")
                nc.sync.dma_start_transpose(
                    out=kt, in_=krot[:, h * 128:(h + 1) * 128])
                d["kt"] = kt
            if "vt" not in d and staged_c[3 * g + 2]:
                vt = pb.tile([128, KT, 128], BF, name="vt", tag="vt")
                nc.sync.dma_start(
                    out=vt,
                    in_=vnat[:, h * 128:(h + 1) * 128].rearrange(
                        "(j p) d -> p j d", p=128))
                d["vt"] = vt
            return len(d) == 3

        def b_heads(hs, pools, tail_load=None):
            pss, psy, psd, paired = pools

            def maybe_load(h2):
                if h2 is not None and len(pend.get(h2, ())) < 3:
                    b_load(h2)

            for idx, h in enumerate(hs):
                nxt = hs[idx + 1] if idx + 1 < len(hs) else tail_load
                while not b_load(h):
                    yield           # spin: let the A stream emit staging
                d = pend.pop(h)
                qt, kt, vt = d["qt"], d["kt"], d["vt"]

                for qn, qc in enumerate([2, 3, 1, 0]):
                    if qn >= 1:
                        maybe_load(nxt)
                    nkt = 4 * qc + 4
                    npair = 2 * qc + 2      # last 2 pairs are diagonal
                    ypsum = psy.tile([128, 512], F32, name="yp", tag="yp")
                    dpsum = psd.tile([128, 512], F32, name="dp", tag="dp")
                    qs = slice(qc * 512, (qc + 1) * 512)
                    ats = {}

                    def moff(j, qc=qc):
                        # leading masked q-columns of k-tile j's scores
                        return max(0, 128 * (j - 4 * qc))

                    def emit(pg, qc=qc, qt=qt, kt=kt, ats=ats):
                        at = pba.tile([128, 2, 512], BF, name="at", tag="at")
                        if paired:
                            scp = pss.tile([128, 2, 512], F32,
                                           name="scp", tag="scp")
                            for s in range(2):
                                j = 2 * pg + s
                                n = 512 - moff(j)
                                nc.tensor.matmul(
                                    scp[:, s, :n],
                                    kt[:, j * 128:(j + 1) * 128],
                                    qt[:, qc * 512 + 512 - n:(qc + 1) * 512],
                                    start=True, stop=True)
                            n1 = 512 - moff(2 * pg + 1)
                            if moff(2 * pg) == 0:
                                # slots contiguous in the 2-bank tile: one
                                # Exp over [0 : 512+n1]
                                flat = 512 + n1
                                nc.scalar.activation(
                                    out=at.rearrange(
                                        "p a b -> p (a b)")[:, :flat],
                                    in_=scp.rearrange(
                                        "p a b -> p (a b)")[:, :flat],
                                    func=mybir.ActivationFunctionType.Exp,
                                    scale=ALPHA)
                            else:
                                for s in range(2):
                                    n = 512 - moff(2 * pg + s)
                                    nc.scalar.activation(
                                        out=at[:, s, :n], in_=scp[:, s, :n],
                                        func=mybir.ActivationFunctionType.Exp,
                                        scale=ALPHA)
                        else:
                            for s in range(2):
                                j = 2 * pg + s
                                n = 512 - moff(j)
                                scp = pss.tile([128, 512], F32, name="scp",
                                               tag="scp")
                                nc.tensor.matmul(
                                    scp[:, :n],
                                    kt[:, j * 128:(j + 1) * 128],
                                    qt[:, qc * 512 + 512 - n:(qc + 1) * 512],
                                    start=True, stop=True)
                                nc.scalar.activation(
                                    out=at[:, s, :n], in_=scp[:, :n],
                                    func=mybir.ActivationFunctionType.Exp,
                                    scale=ALPHA)
                        for s in range(2):
                            j = 2 * pg + s
                            if moff(j) or j == 4 * qc:
                                nc.vector.tensor_mul(
                                    at[:, s, :128], at[:, s, :128], tri)
                        ats[pg] = at

                    sd = None
                    for pg in range(min(la, npair)):
                        emit(pg)
                    for pg in range(npair):
                        if pg + la < npair:
                            emit(pg + la)
                        at = ats.pop(pg)
                        for s in range(2):
                            j = 2 * pg + s
                            off = moff(j)
                            nc.tensor.matmul(
                                ypsum[:, off:], vt[:, j, :],
                                at[:, s, :512 - off],
                                start=(j == 0), stop=(j == nkt - 1))
                        if pg < 2 * qc:          # full pair
                            if pg % 2 == 0:
                                sg = pbs.tile([128, 512], BF,
                                              name="sg", tag="sg")
                                nc.vector.tensor_add(
                                    sg, at[:, 0, :], at[:, 1, :])
                            else:                # merge into quad, then MM
                                nc.vector.tensor_add(sg, sg, at[:, 0, :])
                                nc.vector.tensor_add(sg, sg, at[:, 1, :])
                                nc.tensor.matmul(
                                    dpsum, ones_c, sg,
                                    start=(pg == 1), stop=False)
                        elif pg == 2 * qc:       # diagonal pair 0
                            sd = pbs.tile([128, 512], BF, name="sd", tag="sd")
                            nc.vector.tensor_scalar_add(sd, at[:, 0, :], 0.0)
                            nc.vector.tensor_add(
                                sd[:, 128:], sd[:, 128:], at[:, 1, :384])
                        else:                    # diagonal pair 1
                            nc.vector.tensor_add(
                                sd[:, 256:], sd[:, 256:], at[:, 0, :256])
                            nc.vector.tensor_add(
                                sd[:, 384:], sd[:, 384:], at[:, 1, :128])
                            nc.tensor.matmul(
                                dpsum, ones_c, sd, start=(qc == 0), stop=True)
                        yield
                        maybe_load(nxt)
                    rbs = pbr.tile([128, 512], BF, name="rbs", tag="rbs")
                    with nc.allow_low_precision("softmax recip bf16"):
                        nc.vector.reciprocal(out=rbs, in_=dpsum)
                    yf = pbr.tile([128, 512], BF, name="yf", tag="yf")
                    nc.vector.tensor_mul(yf, ypsum, rbs)
                    nc.vector.tensor_mul(yh[:, h, qs], ypsum, rbs)
                    nc.vector.tensor_sub(yl[:, h, qs], yf, yh[:, h, qs])

        # ---------------- schedule ----------------
        do_a = "A" in phases
        do_b = "B" in phases
        do_c = "C" in phases

        if do_a:
            for _ in a_chunks([0, 1, 2]):       # S1
                pass
            s1stk.close()
            psa_box.append(astk.enter_context(
                tc.tile_pool(name="psA", bufs=2, space="PSUM")))
        if do_a and do_b:
            with tc.tile_pool(name="psS2", bufs=3, space="PSUM") as pss2, \
                 tc.tile_pool(name="psY2", bufs=2, space="PSUM") as psy2, \
                 tc.tile_pool(name="psD2", bufs=1, space="PSUM") as psd2:
                _weave((a_chunks([3, 4, 5]), A_UNIT),
                       (b_heads([0, 1, 2, 3], (pss2, psy2, psd2, False),
                                tail_load=4), B_UNIT))   # S2
        elif do_a:
            for _ in a_chunks([3, 4, 5]):
                pass
        astk.close()                            # free QKV pools / PSUM

        pcw = bstk.enter_context(tc.tile_pool(name="pCw", bufs=1))
        wohp, wolp = [], []
        if do_c:
            for p in range(NH // 2):
                dd = slice(2 * p, 2 * p + 2)
                t1 = pcw.tile([128, 2, L], FP8, name=f"woh{p}", tag=f"woh{p}")
                nc.sync.dma_start(out=t1, in_=woh_d[:, dd, :])
                wohp.append(t1)
                t2 = pcw.tile([128, 2, L], FP8, name=f"wol{p}", tag=f"wol{p}")
                nc.sync.dma_start(out=t2, in_=wol_d[:, dd, :])
                wolp.append(t2)
        pss3 = bstk.enter_context(
            tc.tile_pool(name="psS3", bufs=2, space="PSUM"))
        psy3 = bstk.enter_context(
            tc.tile_pool(name="psY3", bufs=2, space="PSUM"))
        psd3 = bstk.enter_context(
            tc.tile_pool(name="psD3", bufs=2, space="PSUM"))
        if do_b:
            rest = [4, 5, 6, 7] if do_a else list(range(NH))
            for _ in b_heads(rest, (pss3, psy3, psd3, True)):   # S3
                pass

        # ---------------- S4: out-projection (reuses B pools) ----------------
        if do_c:
                for e in range(DT):
                    ec = slice(e * 128, (e + 1) * 128)
                    for qc in range(QC):
                        qs = slice(qc * 512, (qc + 1) * 512)
                        op = pss3.tile([128, 2, 512], F32, name="op",
                                       tag="scp")[:, 0, :]
                        for p in range(NH // 2):
                            dd = slice(2 * p, 2 * p + 2)
                            nc.tensor.matmul(
                                op, wohp[p][:, :, ec], yh[:, dd, qs],
                                start=(p == 0), stop=False, perf_mode=DR)
                            nc.tensor.matmul(
                                op, wohp[p][:, :, ec], yl[:, dd, qs],
                                start=False, stop=False, perf_mode=DR)
                            nc.tensor.matmul(
                                op, wolp[p][:, :, ec], yh[:, dd, qs],
                                start=False, stop=(p == NH // 2 - 1),
                                perf_mode=DR)
                        ot = pba.tile([128, 2, 512], BF, name="ot",
                                      tag="at")[:, 0, :]
                        nc.scalar.copy(out=ot, in_=op)
                        nc.sync.dma_start(
                            out=outT[e * 128:(e + 1) * 128, qs], in_=ot)
        bstk.close()
    nc.compile()
    return nc


_NC_CACHE = None


def _get_program():
    global _NC_CACHE
    if _NC_CACHE is None:
        _NC_CACHE = build_program()
    return _NC_CACHE


def _q8(a):
    return np.clip(a, -240.0, 240.0).astype(E4)


def _hilo(a):
    hi = _q8(a)
    lo = _q8(a - hi.astype(np.float32))
    return hi, lo


def _host_inputs(x, w_qkv, w_o):
    inv = 1.0 / (ROPE_BASE ** (np.arange(0, HD, 2, dtype=np.float64) / HD))
    ang = np.arange(L, dtype=np.float64)[:, None] * inv[None, :]
    chalf = np.tile(np.cos(ang), (1, 4)).astype(BF16)          # [L, 256]
    shalf = np.tile(np.sin(ang), (1, 4)).astype(BF16)
    p = np.arange(128)[:, None]
    f = np.arange(128)[None, :]
    tri = (p <= f).astype(BF16)

    def to_pdl(a, nt):  # [nt*128, cols] -> [128, nt, cols]
        return np.ascontiguousarray(
            a.reshape(nt, 128, a.shape[1]).transpose(1, 0, 2))

    xs = {}
    for b in range(B):
        xh, xl = _hilo(XS * x[b].T)
        xs[b] = (to_pdl(xh, DT), to_pdl(xl, DT))

    in_maps = []
    for c in range(8):
        b, g = c % 4, c // 4
        qr = w_qkv[g * DL:(g + 1) * DL]
        kr = w_qkv[D + g * DL:D + (g + 1) * DL]
        vr = w_qkv[2 * D + g * DL:2 * D + (g + 1) * DL]
        wqkvT = np.concatenate(
            [qr[:512], kr[:512], vr[:512],
             qr[512:], kr[512:], vr[512:]], axis=0).T  # [D, 3DL]
        wh, wl = _hilo(WS * wqkvT)
        woT = w_o[:, g * DL:(g + 1) * DL].T            # [DL, D]
        woh, wol = _hilo(OS * woT)
        in_maps.append({
            "xh": xs[b][0], "xl": xs[b][1],
            "wh": to_pdl(wh, DT), "wl": to_pdl(wl, DT),
            "woh": to_pdl(woh, NH), "wol": to_pdl(wol, NH),
            "chalf": chalf, "shalf": shalf, "tri": tri,
        })
    return in_maps


def kernel(x, w_qkv, w_o, _trace=False):
    x = np.asarray(x, dtype=np.float32)
    w_qkv = np.asarray(w_qkv, dtype=np.float32)
    w_o = np.asarray(w_o, dtype=np.float32)
    nc = _get_program()
    in_maps = _host_inputs(x, w_qkv, w_o)
    res = run_bass_kernel_spmd(nc, in_maps, core_ids=list(range(8)),
                               trace=_trace)
    kernel.last_result = res
    parts = [r["outT"].astype(np.float32) for r in res.results]
    inv_scale = np.float32(1.0 / OUT_SCALE)
    out = np.empty((B, L, D), dtype=np.float32)
    for b in range(B):
        out[b] = (parts[b] + parts[b + 4]).T * inv_scale
    return out


# revision 54
# speedup vs baseline: 1.0052x; 1.0018x over previous
"""MHA (RoPE + causal softmax attention + out-proj) on 8 NeuronCores.

Sharding: DP4 x TP2. Core c: batch b = c % 4, head-group g = c // 4
(8 heads per core). Each core computes a transposed partial output
outT = (y_local @ w_o_slice^T)^T in [D, L]; host sums the two head-group
partials per batch, transposes back and divides by the operand scale.

Precision strategy:
  QKV and out-proj matmuls run as hi/lo-compensated fp8e4 DoubleRow
  (2 k-tiles per MM at 0.5 cycles/row): a = a_hi + a_lo with both parts
  e4m3 and the residual UNSCALED (operands are pre-scaled on the host --
  x by 8, w_qkv by 128, w_o by 64 -- so residuals sit in e4m3's normal
  range). Then a.b ~ a_hi.b_hi + a_hi.b_lo + a_lo.b_hi: all three
  products share one scale and accumulate in a single PSUM (the lo.lo
  term is dropped, ~1e-3 relative). 3 DoubleRow MMs per 2 k-tiles =
  0.75x the bf16 cycle count. Attention itself stays bf16 (fp8 there
  fails the 2e-2 budget; measured).

Schedule (PE executes in emission order, so overlap is explicit):
  S1: QKV chunks q03,k03,v03 (group-0 staging ready at the end)
  S2: QKV chunks q47,k47,v47 interleaved with attention heads 0-3,
      woven by PE-cost so attention's Exp (ScalarE) and softmax DVE work
      hide under the QKV DoubleRow matmul stream
  S3: attention heads 4-7 (QKV pools closed, out-proj weights loaded)
  S4: out-projection (compensated DR fp8)

Attention per head: q/k reloaded transposed via DMA xbar; scores per
k-tile pair into a 2-bank PSUM; causal handled by skipping fully-masked
k-tiles, trimming diagonal tiles, and a [128,128] triangle mask.
Softmax denominator: DVE pair-sums + one ones(=64)-matmul per pair
(no per-k-tile denominator matmuls). y emitted as fp8 hi/lo for S4.
"""

import contextlib

import numpy as np
import ml_dtypes

import concourse.bass as bass
import concourse.tile as tile
import concourse.mybir as mybir
from concourse import bacc
from concourse.bass_utils import run_bass_kernel_spmd

E4 = ml_dtypes.float8_e4m3
BF16 = ml_dtypes.bfloat16
F32 = mybir.dt.float32
BF = mybir.dt.bfloat16
FP8 = mybir.dt.float8e4
DR = mybir.MatmulPerfMode.DoubleRow

B, L, D, H, HD = 4, 2048, 2048, 16, 128
NH = 8                      # heads per core
DL = NH * HD                # 1024 local head dims
ROPE_BASE = 10000.0

XS = 8.0                    # host scale on x
WS = 128.0                  # host scale on w_qkv
OS = 64.0                   # host scale on w_o
ONES_C = 64.0               # denominator constant: y_dev = (XS*WS/ONES_C)*y
OUT_SCALE = (XS * WS / ONES_C) * OS   # 1024: host divides outT by this
ALPHA = float(HD) ** -0.5 / (XS * XS * WS * WS)

LT = L // 128               # 16 L-tiles
DT = D // 128               # 16 D(contract)-tiles
NCH = 6                     # qkv chunks of 512 comps: q03,k03,v03,q47,k47,v47
QC = L // 512               # 4 q-chunks of 512
KT = L // 128               # 16 k-tiles

A_UNIT = 2.56               # relative PE cost of one QKV output tile
B_UNIT = 1.56               # weave weight: spread 4 heads over A's tail


def _chunk_kind(c):
    # chunk order: q(heads0-3), k(0-3), v(0-3), q(4-7), k(4-7), v(4-7)
    return ("q", "k", "v")[c % 3], c // 3


def _weave(*streams):
    """Advance generators round-robin, weighted by per-unit PE cost.

    streams: (generator, unit_cost) pairs. Each next() should emit about
    unit_cost worth of PE work.
    """
    acc = [0.0] * len(streams)
    alive = [True] * len(streams)
    while any(alive):
        k = min((i for i in range(len(streams)) if alive[i]),
                key=lambda i: acc[i])
        try:
            next(streams[k][0])
            acc[k] += streams[k][1]
        except StopIteration:
            alive[k] = False


def build_program(phases="ABC", la=2):
    nc = bacc.Bacc("TRN2", target_bir_lowering=False, debug=False, num_devices=8)

    xh_d = nc.dram_tensor("xh", [128, DT, L], FP8, kind="ExternalInput").ap()
    xl_d = nc.dram_tensor("xl", [128, DT, L], FP8, kind="ExternalInput").ap()
    wh_d = nc.dram_tensor("wh", [128, DT, 3 * DL], FP8, kind="ExternalInput").ap()
    wl_d = nc.dram_tensor("wl", [128, DT, 3 * DL], FP8, kind="ExternalInput").ap()
    woh_d = nc.dram_tensor("woh", [128, NH, L], FP8, kind="ExternalInput").ap()
    wol_d = nc.dram_tensor("wol", [128, NH, L], FP8, kind="ExternalInput").ap()
    chalf = nc.dram_tensor("chalf", [L, 256], BF, kind="ExternalInput").ap()
    shalf = nc.dram_tensor("shalf", [L, 256], BF, kind="ExternalInput").ap()
    tri_d = nc.dram_tensor("tri", [128, 128], BF, kind="ExternalInput").ap()
    outT = nc.dram_tensor("outT", [D, L], BF, kind="ExternalOutput").ap()

    qrot = nc.dram_tensor("qrot", [L, DL], BF, kind="Internal").ap()
    krot = nc.dram_tensor("krot", [L, DL], BF, kind="Internal").ap()
    vnat = nc.dram_tensor("vnat", [L, DL], BF, kind="Internal").ap()

    with tile.TileContext(nc) as tc, contextlib.ExitStack() as stk:
        ex = stk.enter_context
        outer = ex(tc.tile_pool(name="outer", bufs=1))
        pb = ex(tc.tile_pool(name="pBqk", bufs=2, side="right"))
        pbm = ex(tc.tile_pool(name="pBm", bufs=1, side="right"))
        bstk = contextlib.ExitStack()
        pba = bstk.enter_context(tc.tile_pool(name="pBa", bufs=la + 2))
        pbs = bstk.enter_context(tc.tile_pool(name="pBs", bufs=2))
        pbr = bstk.enter_context(tc.tile_pool(name="pBr", bufs=2))

        yh = outer.tile([128, NH, L], FP8, name="yh", tag="yh")
        yl = outer.tile([128, NH, L], FP8, name="yl", tag="yl")
        if "B" not in phases:
            nc.vector.memset(yh, 0.0)
            nc.vector.memset(yl, 0.0)
        ones_c = outer.tile([128, 128], BF, name="ones_c", tag="oc")
        nc.vector.memset(ones_c, ONES_C)
        tri = pbm.tile([128, 128], BF, name="tri", tag="tri")
        nc.sync.dma_start(out=tri, in_=tri_d)

        # -------- Phase A stream: QKV + RoPE (one yield per L-tile) --------
        astk = contextlib.ExitStack()
        pa = astk.enter_context(tc.tile_pool(name="pA", bufs=1))
        paw = astk.enter_context(tc.tile_pool(name="pAw", bufs=2))
        pat = astk.enter_context(tc.tile_pool(name="pAt", bufs=1))
        pao = astk.enter_context(tc.tile_pool(name="pAo", bufs=3))
        psa_box = []

        # per-d-pair x and chunk-0 weight tiles, DMA-interleaved so the
        # first matmuls start after ~4us instead of ~30us
        xhp, xlp, wh0p, wl0p = [], [], [], []
        c_sb = s_sb = None
        for dp in range(DT // 2):
            dd = slice(2 * dp, 2 * dp + 2)
            th = pa.tile([128, 2, L], FP8, name=f"xh{dp}", tag=f"xh{dp}")
            nc.sync.dma_start(out=th, in_=xh_d[:, dd, :])
            xhp.append(th)
            tl = pa.tile([128, 2, L], FP8, name=f"xl{dp}", tag=f"xl{dp}")
            nc.sync.dma_start(out=tl, in_=xl_d[:, dd, :])
            xlp.append(tl)
            twh = pa.tile([128, 2, 512], FP8, name=f"wh0{dp}", tag=f"wh0{dp}")
            nc.sync.dma_start(out=twh, in_=wh_d[:, dd, 0:512])
            wh0p.append(twh)
            twl = pa.tile([128, 2, 512], FP8, name=f"wl0{dp}", tag=f"wl0{dp}")
            nc.sync.dma_start(out=twl, in_=wl_d[:, dd, 0:512])
            wl0p.append(twl)
            if dp == 2:
                c_sb = pa.tile([128, LT, 256], BF, name="c_sb", tag="c_sb")
                nc.sync.dma_start(
                    out=c_sb, in_=chalf.rearrange("(i p) g -> p i g", p=128))
                s_sb = pa.tile([128, LT, 256], BF, name="s_sb", tag="s_sb")
                nc.sync.dma_start(
                    out=s_sb, in_=shalf.rearrange("(i p) g -> p i g", p=128))

        def rope_or_v(kind, grp, i, pnat):
            if kind == "v":
                vo = pao.tile([128, 512], BF, name="vo", tag="ro")
                nc.scalar.copy(out=vo, in_=pnat)
                nc.sync.dma_start(
                    out=vnat[i * 128:(i + 1) * 128,
                             grp * 512:(grp + 1) * 512],
                    in_=vo)
            else:
                x1 = pnat[:, 0::2]
                x2 = pnat[:, 1::2]
                ct = c_sb[:, i, :]
                st = s_sb[:, i, :]
                t1 = pat.tile([128, 256], F32, name="t1", tag="t1")
                nc.vector.tensor_mul(t1, x1, ct)
                t2 = pat.tile([128, 256], F32, name="t2", tag="t2")
                nc.vector.tensor_mul(t2, x2, st)
                t3 = pat.tile([128, 256], F32, name="t3", tag="t3")
                nc.vector.tensor_mul(t3, x2, ct)
                t4 = pat.tile([128, 256], F32, name="t4", tag="t4")
                nc.vector.tensor_mul(t4, x1, st)
                ro = pao.tile([128, 512], BF, name="ro", tag="ro")
                nc.vector.tensor_sub(ro[:, 0::2], t1, t2)
                nc.vector.tensor_add(ro[:, 1::2], t3, t4)
                dst = qrot if kind == "q" else krot
                nc.sync.dma_start(
                    out=dst[i * 128:(i + 1) * 128,
                            grp * 512:(grp + 1) * 512],
                    in_=ro)

        s1stk = contextlib.ExitStack()

        def a_chunk0():
            # dp-outer over L-halves with 8 PSUM banks: matmuls consume
            # x/w d-pairs as their DMAs land instead of waiting for all 16
            kind, grp = _chunk_kind(0)
            ps0 = s1stk.enter_context(
                tc.tile_pool(name="ps0", bufs=1, space="PSUM"))
            psa_box.append(ps0)
            for half in range(2):
                pns = [ps0.tile([128, 512], F32, name=f"pn{ii}",
                                tag=f"pn{ii}") for ii in range(8)]
                for dp in range(DT // 2):
                    for ii in range(8):
                        i = half * 8 + ii
                        ic = slice(i * 128, (i + 1) * 128)
                        nc.tensor.matmul(
                            pns[ii], xhp[dp][:, :, ic], wh0p[dp],
                            start=(dp == 0), stop=False, perf_mode=DR)
                        nc.tensor.matmul(
                            pns[ii], xhp[dp][:, :, ic], wl0p[dp],
                            start=False, stop=False, perf_mode=DR)
                        nc.tensor.matmul(
                            pns[ii], xlp[dp][:, :, ic], wh0p[dp],
                            start=False, stop=(dp == DT // 2 - 1),
                            perf_mode=DR)
                for ii in range(8):
                    rope_or_v(kind, grp, half * 8 + ii, pns[ii])
                    yield

        staged_c = [False] * NCH  # chunk-c staging stores all emitted

        def a_chunks(cs):
            for c in cs:
                kind, grp = _chunk_kind(c)
                if c == 0:
                    yield from a_chunk0()
                    staged_c[0] = True
                    continue
                else:
                    wh = paw.tile([128, DT, 512], FP8, name="wh", tag="wh")
                    nc.sync.dma_start(
                        out=wh, in_=wh_d[:, :, c * 512:(c + 1) * 512])
                    wl = paw.tile([128, DT, 512], FP8, name="wl", tag="wl")
                    nc.sync.dma_start(
                        out=wl, in_=wl_d[:, :, c * 512:(c + 1) * 512])
                    whs = [wh[:, slice(2 * dp, 2 * dp + 2), :]
                           for dp in range(DT // 2)]
                    wls = [wl[:, slice(2 * dp, 2 * dp + 2), :]
                           for dp in range(DT // 2)]
                psa = psa_box[-1]
                for i in range(LT):
                    if c < 3:
                        pnat = psa.tile([128, 512], F32, name=f"pn{i % 8}",
                                        tag=f"pn{i % 8}")
                    else:
                        pnat = psa.tile([128, 512], F32, name="pnat",
                                        tag="pnat")
                    ic = slice(i * 128, (i + 1) * 128)
                    for dp in range(DT // 2):
                        nc.tensor.matmul(
                            pnat, xhp[dp][:, :, ic], whs[dp],
                            start=(dp == 0), stop=False, perf_mode=DR)
                        nc.tensor.matmul(
                            pnat, xhp[dp][:, :, ic], wls[dp],
                            start=False, stop=False, perf_mode=DR)
                        nc.tensor.matmul(
                            pnat, xlp[dp][:, :, ic], whs[dp],
                            start=False, stop=(dp == DT // 2 - 1),
                            perf_mode=DR)
                    rope_or_v(kind, grp, i, pnat)
                    yield
                staged_c[c] = True

        # -------- Phase B stream: attention (one yield per pair-group) ----
        pend = {}

        def b_load(h):
            # gated per source chunk: emit each load as soon as its staging
            # chunk is fully emitted; returns True when all three are in
            g = h // 4
            d = pend.setdefault(h, {})
            if "qt" not in d and staged_c[3 * g]:
                qt = pb.tile([128, L], BF, name="qt", tag="qt")
                nc.sync.dma_start_transpose(
                    out=qt, in_=qrot[:, h * 128:(h + 1) * 128])
                d["qt"] = qt
            if "kt" not in d and staged_c[3 * g + 1]:
                kt = pb.tile([128, L], BF, name="kt", tag="kt")
                nc.sync.dma_start_transpose(
                    out=kt, in_=krot[:, h * 128:(h + 1) * 128])
                d["kt"] = kt
            if "vt" not in d and staged_c[3 * g + 2]:
                vt = pb.tile([128, KT, 128], BF, name="vt", tag="vt")
                nc.sync.dma_start(
                    out=vt,
                    in_=vnat[:, h * 128:(h + 1) * 128].rearrange(
                        "(j p) d -> p j d", p=128))
                d["vt"] = vt
            return len(d) == 3

        def b_heads(hs, pools, tail_load=None):
            pss, psy, psd, paired = pools

            def maybe_load(h2):
                if h2 is not None and len(pend.get(h2, ())) < 3:
                    b_load(h2)

            for idx, h in enumerate(hs):
                nxt = hs[idx + 1] if idx + 1 < len(hs) else tail_load
                while not b_load(h):
                    yield           # spin: let the A stream emit staging
                d = pend.pop(h)
                qt, kt, vt = d["qt"], d["kt"], d["vt"]

                for qn, qc in enumerate([1, 3, 2, 0]):
                    if qn >= 1:
                        maybe_load(nxt)
                    nkt = 4 * qc + 4
                    npair = 2 * qc + 2      # last 2 pairs are diagonal
                    ypsum = psy.tile([128, 512], F32, name="yp", tag="yp")
                    dpsum = psd.tile([128, 512], F32, name="dp", tag="dp")
                    qs = slice(qc * 512, (qc + 1) * 512)
                    ats = {}

                    def moff(j, qc=qc):
                        # leading masked q-columns of k-tile j's scores
                        return max(0, 128 * (j - 4 * qc))

                    def emit(pg, qc=qc, qt=qt, kt=kt, ats=ats):
                        at = pba.tile([128, 2, 512], BF, name="at", tag="at")
                        if paired:
                            scp = pss.tile([128, 2, 512], F32,
                                           name="scp", tag="scp")
                            for s in range(2):
                                j = 2 * pg + s
                                n = 512 - moff(j)
                                nc.tensor.matmul(
                                    scp[:, s, :n],
                                    kt[:, j * 128:(j + 1) * 128],
                                    qt[:, qc * 512 + 512 - n:(qc + 1) * 512],
                                    start=True, stop=True)
                            n1 = 512 - moff(2 * pg + 1)
                            if moff(2 * pg) == 0:
                                # slots contiguous in the 2-bank tile: one
                                # Exp over [0 : 512+n1]
                                flat = 512 + n1
                                nc.scalar.activation(
                                    out=at.rearrange(
                                        "p a b -> p (a b)")[:, :flat],
                                    in_=scp.rearrange(
                                        "p a b -> p (a b)")[:, :flat],
                                    func=mybir.ActivationFunctionType.Exp,
                                    scale=ALPHA)
                            else:
                                for s in range(2):
                                    n = 512 - moff(2 * pg + s)
                                    nc.scalar.activation(
                                        out=at[:, s, :n], in_=scp[:, s, :n],
                                        func=mybir.ActivationFunctionType.Exp,
                                        scale=ALPHA)
                        else:
                            for s in range(2):
                                j = 2 * pg + s
                                n = 512 - moff(j)
                                scp = pss.tile([128, 512], F32, name="scp",
                                               tag="scp")
                                nc.tensor.matmul(
                                    scp[:, :n],
                                    kt[:, j * 128:(j + 1) * 128],
                                    qt[:, qc * 512 + 512 - n:(qc + 1) * 512],
                                    start=True, stop=True)
                                nc.scalar.activation(
                                    out=at[:, s, :n], in_=scp[:, :n],
                                    func=mybir.ActivationFunctionType.Exp,
                                    scale=ALPHA)
                        for s in range(2):
                            j = 2 * pg + s
                            if moff(j) or j == 4 * qc:
                                nc.vector.tensor_mul(
                                    at[:, s, :128], at[:, s, :128], tri)
                        ats[pg] = at

                    sd = None
                    for pg in range(min(la, npair)):
                        emit(pg)
                    for pg in range(npair):
                        if pg + la < npair:
                            emit(pg + la)
                        at = ats.pop(pg)
                        for s in range(2):
                            j = 2 * pg + s
                            off = moff(j)
                            nc.tensor.matmul(
                                ypsum[:, off:], vt[:, j, :],
                                at[:, s, :512 - off],
                                start=(j == 0), stop=(j == nkt - 1))
                        if pg < 2 * qc:          # full pair
                            if pg % 2 == 0:
                                sg = pbs.tile([128, 512], BF,
                                              name="sg", tag="sg")
                                nc.vector.tensor_add(
                                    sg, at[:, 0, :], at[:, 1, :])
                            else:                # merge into quad, then MM
                                nc.vector.tensor_add(sg, sg, at[:, 0, :])
                                nc.vector.tensor_add(sg, sg, at[:, 1, :])
                                nc.tensor.matmul(
                                    dpsum, ones_c, sg,
                                    start=(pg == 1), stop=False)
                        elif pg == 2 * qc:       # diagonal pair 0
                            sd = pbs.tile([128, 512], BF, name="sd", tag="sd")
                            nc.vector.tensor_scalar_add(sd, at[:, 0, :], 0.0)
                            nc.vector.tensor_add(
                                sd[:, 128:], sd[:, 128:], at[:, 1, :384])
                        else:                    # diagonal pair 1
                            nc.vector.tensor_add(
                                sd[:, 256:], sd[:, 256:], at[:, 0, :256])
                            nc.vector.tensor_add(
                                sd[:, 384:], sd[:, 384:], at[:, 1, :128])
                            nc.tensor.matmul(
                                dpsum, ones_c, sd, start=(qc == 0), stop=True)
                        yield
                        maybe_load(nxt)
                    rbs = pbr.tile([128, 512], BF, name="rbs", tag="rbs")
                    with nc.allow_low_precision("softmax recip bf16"):
                        nc.vector.reciprocal(out=rbs, in_=dpsum)
                    yf = pbr.tile([128, 512], BF, name="yf", tag="yf")
                    nc.vector.tensor_mul(yf, ypsum, rbs)
                    nc.vector.tensor_mul(yh[:, h, qs], ypsum, rbs)
                    nc.vector.tensor_sub(yl[:, h, qs], yf, yh[:, h, qs])

        # ---------------- schedule ----------------
        do_a = "A" in phases
        do_b = "B" in phases
        do_c = "C" in phases

        if do_a:
            for _ in a_chunks([0, 1, 2]):       # S1
                pass
            s1stk.close()
            psa_box.append(astk.enter_context(
                tc.tile_pool(name="psA", bufs=2, space="PSUM")))
        if do_a and do_b:
            with tc.tile_pool(name="psS2", bufs=3, space="PSUM") as pss2, \
                 tc.tile_pool(name="psY2", bufs=2, space="PSUM") as psy2, \
                 tc.tile_pool(name="psD2", bufs=1, space="PSUM") as psd2:
                _weave((a_chunks([3, 4, 5]), A_UNIT),
                       (b_heads([0, 1, 2, 3], (pss2, psy2, psd2, False),
                                tail_load=4), B_UNIT))   # S2
        elif do_a:
            for _ in a_chunks([3, 4, 5]):
                pass
        astk.close()                            # free QKV pools / PSUM

        pcw = bstk.enter_context(tc.tile_pool(name="pCw", bufs=1))
        wohp, wolp = [], []
        if do_c:
            for p in range(NH // 2):
                dd = slice(2 * p, 2 * p + 2)
                t1 = pcw.tile([128, 2, L], FP8, name=f"woh{p}", tag=f"woh{p}")
                nc.sync.dma_start(out=t1, in_=woh_d[:, dd, :])
                wohp.append(t1)
                t2 = pcw.tile([128, 2, L], FP8, name=f"wol{p}", tag=f"wol{p}")
                nc.sync.dma_start(out=t2, in_=wol_d[:, dd, :])
                wolp.append(t2)
        pss3 = bstk.enter_context(
            tc.tile_pool(name="psS3", bufs=2, space="PSUM"))
        psy3 = bstk.enter_context(
            tc.tile_pool(name="psY3", bufs=2, space="PSUM"))
        psd3 = bstk.enter_context(
            tc.tile_pool(name="psD3", bufs=2, space="PSUM"))
        if do_b:
            rest = [4, 5, 6, 7] if do_a else list(range(NH))
            for _ in b_heads(rest, (pss3, psy3, psd3, True)):   # S3
                pass

        # ---------------- S4: out-projection (reuses B pools) ----------------
        if do_c:
                for e in range(DT):
                    ec = slice(e * 128, (e + 1) * 128)
                    for qc in range(QC):
                        qs = slice(qc * 512, (qc + 1) * 512)
                        op = pss3.tile([128, 2, 512], F32, name="op",
                                       tag="scp")[:, 0, :]
                        for p in range(NH // 2):
                            dd = slice(2 * p, 2 * p + 2)
                            nc.tensor.matmul(
                                op, wohp[p][:, :, ec], yh[:, dd, qs],
                                start=(p == 0), stop=False, perf_mode=DR)
                            nc.tensor.matmul(
                                op, wohp[p][:, :, ec], yl[:, dd, qs],
                                start=False, stop=False, perf_mode=DR)
                            nc.tensor.matmul(
                                op, wolp[p][:, :, ec], yh[:, dd, qs],
                                start=False, stop=(p == NH // 2 - 1),
                                perf_mode=DR)
                        ot = pba.tile([128, 2, 512], BF, name="ot",
                                      tag="at")[:, 0, :]
                        nc.scalar.copy(out=ot, in_=op)
                        nc.sync.dma_start(
                            out=outT[e * 128:(e + 1) * 128, qs], in_=ot)
        bstk.close()
    nc.compile()
    return nc


_NC_CACHE = None


def _get_program():
    global _NC_CACHE
    if _NC_CACHE is None:
        _NC_CACHE = build_program()
    return _NC_CACHE


def _q8(a):
    return np.clip(a, -240.0, 240.0).astype(E4)


def _hilo(a):
    hi = _q8(a)
    lo = _q8(a - hi.astype(np.float32))
    return hi, lo


def _host_inputs(x, w_qkv, w_o):
    inv = 1.0 / (ROPE_BASE ** (np.arange(0, HD, 2, dtype=np.float64) / HD))
    ang = np.arange(L, dtype=np.float64)[:, None] * inv[None, :]
    chalf = np.tile(np.cos(ang), (1, 4)).astype(BF16)          # [L, 256]
    shalf = np.tile(np.sin(ang), (1, 4)).astype(BF16)
    p = np.arange(128)[:, None]
    f = np.arange(128)[None, :]
    tri = (p <= f).astype(BF16)

    def to_pdl(a, nt):  # [nt*128, cols] -> [128, nt, cols]
        return np.ascontiguousarray(
            a.reshape(nt, 128, a.shape[1]).transpose(1, 0, 2))

    xs = {}
    for b in range(B):
        xh, xl = _hilo(XS * x[b].T)
        xs[b] = (to_pdl(xh, DT), to_pdl(xl, DT))

    in_maps = []
    for c in range(8):
        b, g = c % 4, c // 4
        qr = w_qkv[g * DL:(g + 1) * DL]
        kr = w_qkv[D + g * DL:D + (g + 1) * DL]
        vr = w_qkv[2 * D + g * DL:2 * D + (g + 1) * DL]
        wqkvT = np.concatenate(
            [qr[:512], kr[:512], vr[:512],
             qr[512:], kr[512:], vr[512:]], axis=0).T  # [D, 3DL]
        wh, wl = _hilo(WS * wqkvT)
        woT = w_o[:, g * DL:(g + 1) * DL].T            # [DL, D]
        woh, wol = _hilo(OS * woT)
        in_maps.append({
            "xh": xs[b][0], "xl": xs[b][1],
            "wh": to_pdl(wh, DT), "wl": to_pdl(wl, DT),
            "woh": to_pdl(woh, NH), "wol": to_pdl(wol, NH),
            "chalf": chalf, "shalf": shalf, "tri": tri,
        })
    return in_maps


def kernel(x, w_qkv, w_o, _trace=False):
    x = np.asarray(x, dtype=np.float32)
    w_qkv = np.asarray(w_qkv, dtype=np.float32)
    w_o = np.asarray(w_o, dtype=np.float32)
    nc = _get_program()
    in_maps = _host_inputs(x, w_qkv, w_o)
    res = run_bass_kernel_spmd(nc, in_maps, core_ids=list(range(8)),
                               trace=_trace)
    kernel.last_result = res
    parts = [r["outT"].astype(np.float32) for r in res.results]
    inv_scale = np.float32(1.0 / OUT_SCALE)
    out = np.empty((B, L, D), dtype=np.float32)
    for b in range(B):
        out[b] = (parts[b] + parts[b + 4]).T * inv_scale
    return out


# revision 55
# speedup vs baseline: 1.0089x; 1.0037x over previous
"""MHA (RoPE + causal softmax attention + out-proj) on 8 NeuronCores.

Sharding: DP4 x TP2. Core c: batch b = c % 4, head-group g = c // 4
(8 heads per core). Each core computes a transposed partial output
outT = (y_local @ w_o_slice^T)^T in [D, L]; host sums the two head-group
partials per batch, transposes back and divides by the operand scale.

Precision strategy:
  QKV and out-proj matmuls run as hi/lo-compensated fp8e4 DoubleRow
  (2 k-tiles per MM at 0.5 cycles/row): a = a_hi + a_lo with both parts
  e4m3 and the residual UNSCALED (operands are pre-scaled on the host --
  x by 8, w_qkv by 128, w_o by 64 -- so residuals sit in e4m3's normal
  range). Then a.b ~ a_hi.b_hi + a_hi.b_lo + a_lo.b_hi: all three
  products share one scale and accumulate in a single PSUM (the lo.lo
  term is dropped, ~1e-3 relative). 3 DoubleRow MMs per 2 k-tiles =
  0.75x the bf16 cycle count. Attention itself stays bf16 (fp8 there
  fails the 2e-2 budget; measured).

Schedule (PE executes in emission order, so overlap is explicit):
  S1: QKV chunks q03,k03,v03 (group-0 staging ready at the end)
  S2: QKV chunks q47,k47,v47 interleaved with attention heads 0-3,
      woven by PE-cost so attention's Exp (ScalarE) and softmax DVE work
      hide under the QKV DoubleRow matmul stream
  S3: attention heads 4-7 (QKV pools closed, out-proj weights loaded)
  S4: out-projection (compensated DR fp8)

Attention per head: q/k reloaded transposed via DMA xbar; scores per
k-tile pair into a 2-bank PSUM; causal handled by skipping fully-masked
k-tiles, trimming diagonal tiles, and a [128,128] triangle mask.
Softmax denominator: DVE pair-sums + one ones(=64)-matmul per pair
(no per-k-tile denominator matmuls). y emitted as fp8 hi/lo for S4.
"""

import contextlib

import numpy as np
import ml_dtypes

import concourse.bass as bass
import concourse.tile as tile
import concourse.mybir as mybir
from concourse import bacc
from concourse.bass_utils import run_bass_kernel_spmd

E4 = ml_dtypes.float8_e4m3
BF16 = ml_dtypes.bfloat16
F32 = mybir.dt.float32
BF = mybir.dt.bfloat16
FP8 = mybir.dt.float8e4
DR = mybir.MatmulPerfMode.DoubleRow

B, L, D, H, HD = 4, 2048, 2048, 16, 128
NH = 8                      # heads per core
DL = NH * HD                # 1024 local head dims
ROPE_BASE = 10000.0

XS = 8.0                    # host scale on x
WS = 128.0                  # host scale on w_qkv
OS = 64.0                   # host scale on w_o
ONES_C = 64.0               # denominator constant: y_dev = (XS*WS/ONES_C)*y
OUT_SCALE = (XS * WS / ONES_C) * OS   # 1024: host divides outT by this
ALPHA = float(HD) ** -0.5 / (XS * XS * WS * WS)

LT = L // 128               # 16 L-tiles
DT = D // 128               # 16 D(contract)-tiles
NCH = 6                     # qkv chunks of 512 comps: q03,k03,v03,q47,k47,v47
QC = L // 512               # 4 q-chunks of 512
KT = L // 128               # 16 k-tiles

A_UNIT = 2.56               # relative PE cost of one QKV output tile
B_UNIT = 1.64               # weave weight: spread 4 heads over A's tail


def _chunk_kind(c):
    # chunk order: q(heads0-3), k(0-3), v(0-3), q(4-7), k(4-7), v(4-7)
    return ("q", "k", "v")[c % 3], c // 3


def _weave(*streams):
    """Advance generators round-robin, weighted by per-unit PE cost.

    streams: (generator, unit_cost) pairs. Each next() should emit about
    unit_cost worth of PE work.
    """
    acc = [0.0] * len(streams)
    alive = [True] * len(streams)
    while any(alive):
        k = min((i for i in range(len(streams)) if alive[i]),
                key=lambda i: acc[i])
        try:
            next(streams[k][0])
            acc[k] += streams[k][1]
        except StopIteration:
            alive[k] = False


def build_program(phases="ABC", la=2):
    nc = bacc.Bacc("TRN2", target_bir_lowering=False, debug=False, num_devices=8)

    xh_d = nc.dram_tensor("xh", [128, DT, L], FP8, kind="ExternalInput").ap()
    xl_d = nc.dram_tensor("xl", [128, DT, L], FP8, kind="ExternalInput").ap()
    wh_d = nc.dram_tensor("wh", [128, DT, 3 * DL], FP8, kind="ExternalInput").ap()
    wl_d = nc.dram_tensor("wl", [128, DT, 3 * DL], FP8, kind="ExternalInput").ap()
    woh_d = nc.dram_tensor("woh", [128, NH, L], FP8, kind="ExternalInput").ap()
    wol_d = nc.dram_tensor("wol", [128, NH, L], FP8, kind="ExternalInput").ap()
    chalf = nc.dram_tensor("chalf", [L, 256], BF, kind="ExternalInput").ap()
    shalf = nc.dram_tensor("shalf", [L, 256], BF, kind="ExternalInput").ap()
    tri_d = nc.dram_tensor("tri", [128, 128], BF, kind="ExternalInput").ap()
    outT = nc.dram_tensor("outT", [D, L], BF, kind="ExternalOutput").ap()

    qrot = nc.dram_tensor("qrot", [L, DL], BF, kind="Internal").ap()
    krot = nc.dram_tensor("krot", [L, DL], BF, kind="Internal").ap()
    vnat = nc.dram_tensor("vnat", [L, DL], BF, kind="Internal").ap()

    with tile.TileContext(nc) as tc, contextlib.ExitStack() as stk:
        ex = stk.enter_context
        outer = ex(tc.tile_pool(name="outer", bufs=1))
        pb = ex(tc.tile_pool(name="pBqk", bufs=2, side="right"))
        pbm = ex(tc.tile_pool(name="pBm", bufs=1, side="right"))
        bstk = contextlib.ExitStack()
        pba = bstk.enter_context(tc.tile_pool(name="pBa", bufs=la + 2))
        pbs = bstk.enter_context(tc.tile_pool(name="pBs", bufs=2))
        pbr = bstk.enter_context(tc.tile_pool(name="pBr", bufs=2))

        yh = outer.tile([128, NH, L], FP8, name="yh", tag="yh")
        yl = outer.tile([128, NH, L], FP8, name="yl", tag="yl")
        if "B" not in phases:
            nc.vector.memset(yh, 0.0)
            nc.vector.memset(yl, 0.0)
        ones_c = outer.tile([128, 128], BF, name="ones_c", tag="oc")
        nc.vector.memset(ones_c, ONES_C)
        tri = pbm.tile([128, 128], BF, name="tri", tag="tri")
        nc.sync.dma_start(out=tri, in_=tri_d)

        # -------- Phase A stream: QKV + RoPE (one yield per L-tile) --------
        astk = contextlib.ExitStack()
        pa = astk.enter_context(tc.tile_pool(name="pA", bufs=1))
        paw = astk.enter_context(tc.tile_pool(name="pAw", bufs=2))
        pat = astk.enter_context(tc.tile_pool(name="pAt", bufs=1))
        pao = astk.enter_context(tc.tile_pool(name="pAo", bufs=3))
        psa_box = []

        # per-d-pair x and chunk-0 weight tiles, DMA-interleaved so the
        # first matmuls start after ~4us instead of ~30us
        xhp, xlp, wh0p, wl0p = [], [], [], []
        c_sb = s_sb = None
        for dp in range(DT // 2):
            dd = slice(2 * dp, 2 * dp + 2)
            th = pa.tile([128, 2, L], FP8, name=f"xh{dp}", tag=f"xh{dp}")
            nc.sync.dma_start(out=th, in_=xh_d[:, dd, :])
            xhp.append(th)
            tl = pa.tile([128, 2, L], FP8, name=f"xl{dp}", tag=f"xl{dp}")
            nc.sync.dma_start(out=tl, in_=xl_d[:, dd, :])
            xlp.append(tl)
            twh = pa.tile([128, 2, 512], FP8, name=f"wh0{dp}", tag=f"wh0{dp}")
            nc.sync.dma_start(out=twh, in_=wh_d[:, dd, 0:512])
            wh0p.append(twh)
            twl = pa.tile([128, 2, 512], FP8, name=f"wl0{dp}", tag=f"wl0{dp}")
            nc.sync.dma_start(out=twl, in_=wl_d[:, dd, 0:512])
            wl0p.append(twl)
            if dp == 2:
                c_sb = pa.tile([128, LT, 256], BF, name="c_sb", tag="c_sb")
                nc.sync.dma_start(
                    out=c_sb, in_=chalf.rearrange("(i p) g -> p i g", p=128))
                s_sb = pa.tile([128, LT, 256], BF, name="s_sb", tag="s_sb")
                nc.sync.dma_start(
                    out=s_sb, in_=shalf.rearrange("(i p) g -> p i g", p=128))

        def rope_or_v(kind, grp, i, pnat):
            if kind == "v":
                vo = pao.tile([128, 512], BF, name="vo", tag="ro")
                nc.scalar.copy(out=vo, in_=pnat)
                nc.sync.dma_start(
                    out=vnat[i * 128:(i + 1) * 128,
                             grp * 512:(grp + 1) * 512],
                    in_=vo)
            else:
                x1 = pnat[:, 0::2]
                x2 = pnat[:, 1::2]
                ct = c_sb[:, i, :]
                st = s_sb[:, i, :]
                t1 = pat.tile([128, 256], F32, name="t1", tag="t1")
                nc.vector.tensor_mul(t1, x1, ct)
                t2 = pat.tile([128, 256], F32, name="t2", tag="t2")
                nc.vector.tensor_mul(t2, x2, st)
                t3 = pat.tile([128, 256], F32, name="t3", tag="t3")
                nc.vector.tensor_mul(t3, x2, ct)
                t4 = pat.tile([128, 256], F32, name="t4", tag="t4")
                nc.vector.tensor_mul(t4, x1, st)
                ro = pao.tile([128, 512], BF, name="ro", tag="ro")
                nc.vector.tensor_sub(ro[:, 0::2], t1, t2)
                nc.vector.tensor_add(ro[:, 1::2], t3, t4)
                dst = qrot if kind == "q" else krot
                nc.sync.dma_start(
                    out=dst[i * 128:(i + 1) * 128,
                            grp * 512:(grp + 1) * 512],
                    in_=ro)

        s1stk = contextlib.ExitStack()

        def a_chunk0():
            # dp-outer over L-halves with 8 PSUM banks: matmuls consume
            # x/w d-pairs as their DMAs land instead of waiting for all 16
            kind, grp = _chunk_kind(0)
            ps0 = s1stk.enter_context(
                tc.tile_pool(name="ps0", bufs=1, space="PSUM"))
            psa_box.append(ps0)
            for half in range(2):
                pns = [ps0.tile([128, 512], F32, name=f"pn{ii}",
                                tag=f"pn{ii}") for ii in range(8)]
                for dp in range(DT // 2):
                    for ii in range(8):
                        i = half * 8 + ii
                        ic = slice(i * 128, (i + 1) * 128)
                        nc.tensor.matmul(
                            pns[ii], xhp[dp][:, :, ic], wh0p[dp],
                            start=(dp == 0), stop=False, perf_mode=DR)
                        nc.tensor.matmul(
                            pns[ii], xhp[dp][:, :, ic], wl0p[dp],
                            start=False, stop=False, perf_mode=DR)
                        nc.tensor.matmul(
                            pns[ii], xlp[dp][:, :, ic], wh0p[dp],
                            start=False, stop=(dp == DT // 2 - 1),
                            perf_mode=DR)
                for ii in range(8):
                    rope_or_v(kind, grp, half * 8 + ii, pns[ii])
                    yield

        staged_c = [False] * NCH  # chunk-c staging stores all emitted

        def a_chunks(cs):
            for c in cs:
                kind, grp = _chunk_kind(c)
                if c == 0:
                    yield from a_chunk0()
                    staged_c[0] = True
                    continue
                else:
                    wh = paw.tile([128, DT, 512], FP8, name="wh", tag="wh")
                    nc.sync.dma_start(
                        out=wh, in_=wh_d[:, :, c * 512:(c + 1) * 512])
                    wl = paw.tile([128, DT, 512], FP8, name="wl", tag="wl")
                    nc.sync.dma_start(
                        out=wl, in_=wl_d[:, :, c * 512:(c + 1) * 512])
                    whs = [wh[:, slice(2 * dp, 2 * dp + 2), :]
                           for dp in range(DT // 2)]
                    wls = [wl[:, slice(2 * dp, 2 * dp + 2), :]
                           for dp in range(DT // 2)]
                psa = psa_box[-1]
                for i in range(LT):
                    if c < 3:
                        pnat = psa.tile([128, 512], F32, name=f"pn{i % 8}",
                                        tag=f"pn{i % 8}")
                    else:
                        pnat = psa.tile([128, 512], F32, name="pnat",
                                        tag="pnat")
                    ic = slice(i * 128, (i + 1) * 128)
                    for dp in range(DT // 2):
                        nc.tensor.matmul(
                            pnat, xhp[dp][:, :, ic], whs[dp],
                            start=(dp == 0), stop=False, perf_mode=DR)
                        nc.tensor.matmul(
                            pnat, xhp[dp][:, :, ic], wls[dp],
                            start=False, stop=False, perf_mode=DR)
                        nc.tensor.matmul(
                            pnat, xlp[dp][:, :, ic], whs[dp],
                            start=False, stop=(dp == DT // 2 - 1),
                            perf_mode=DR)
                    rope_or_v(kind, grp, i, pnat)
                    yield
                staged_c[c] = True

        # -------- Phase B stream: attention (one yield per pair-group) ----
        pend = {}

        def b_load(h):
            # gated per source chunk: emit each load as soon as its staging
            # chunk is fully emitted; returns True when all three are in
            g = h // 4
            d = pend.setdefault(h, {})
            if "qt" not in d and staged_c[3 * g]:
                qt = pb.tile([128, L], BF, name="qt", tag="qt")
                nc.sync.dma_start_transpose(
                    out=qt, in_=qrot[:, h * 128:(h + 1) * 128])
                d["qt"] = qt
            if "kt" not in d and staged_c[3 * g + 1]:
                kt = pb.tile([128, L], BF, name="kt", tag="kt")
                nc.sync.dma_start_transpose(
                    out=kt, in_=krot[:, h * 128:(h + 1) * 128])
                d["kt"] = kt
            if "vt" not in d and staged_c[3 * g + 2]:
                vt = pb.tile([128, KT, 128], BF, name="vt", tag="vt")
                nc.sync.dma_start(
                    out=vt,
                    in_=vnat[:, h * 128:(h + 1) * 128].rearrange(
                        "(j p) d -> p j d", p=128))
                d["vt"] = vt
            return len(d) == 3

        def b_heads(hs, pools, tail_load=None):
            pss, psy, psd, paired = pools

            def maybe_load(h2):
                if h2 is not None and len(pend.get(h2, ())) < 3:
                    b_load(h2)

            for idx, h in enumerate(hs):
                nxt = hs[idx + 1] if idx + 1 < len(hs) else tail_load
                while not b_load(h):
                    yield           # spin: let the A stream emit staging
                d = pend.pop(h)
                qt, kt, vt = d["qt"], d["kt"], d["vt"]

                for qn, qc in enumerate([1, 3, 2, 0]):
                    if qn >= 1:
                        maybe_load(nxt)
                    nkt = 4 * qc + 4
                    npair = 2 * qc + 2      # last 2 pairs are diagonal
                    ypsum = psy.tile([128, 512], F32, name="yp", tag="yp")
                    dpsum = psd.tile([128, 512], F32, name="dp", tag="dp")
                    qs = slice(qc * 512, (qc + 1) * 512)
                    ats = {}

                    def moff(j, qc=qc):
                        # leading masked q-columns of k-tile j's scores
                        return max(0, 128 * (j - 4 * qc))

                    def emit(pg, qc=qc, qt=qt, kt=kt, ats=ats):
                        at = pba.tile([128, 2, 512], BF, name="at", tag="at")
                        if paired:
                            scp = pss.tile([128, 2, 512], F32,
                                           name="scp", tag="scp")
                            for s in range(2):
                                j = 2 * pg + s
                                n = 512 - moff(j)
                                nc.tensor.matmul(
                                    scp[:, s, :n],
                                    kt[:, j * 128:(j + 1) * 128],
                                    qt[:, qc * 512 + 512 - n:(qc + 1) * 512],
                                    start=True, stop=True)
                            n1 = 512 - moff(2 * pg + 1)
                            if moff(2 * pg) == 0:
                                # slots contiguous in the 2-bank tile: one
                                # Exp over [0 : 512+n1]
                                flat = 512 + n1
                                nc.scalar.activation(
                                    out=at.rearrange(
                                        "p a b -> p (a b)")[:, :flat],
                                    in_=scp.rearrange(
                                        "p a b -> p (a b)")[:, :flat],
                                    func=mybir.ActivationFunctionType.Exp,
                                    scale=ALPHA)
                            else:
                                for s in range(2):
                                    n = 512 - moff(2 * pg + s)
                                    nc.scalar.activation(
                                        out=at[:, s, :n], in_=scp[:, s, :n],
                                        func=mybir.ActivationFunctionType.Exp,
                                        scale=ALPHA)
                        else:
                            for s in range(2):
                                j = 2 * pg + s
                                n = 512 - moff(j)
                                scp = pss.tile([128, 512], F32, name="scp",
                                               tag="scp")
                                nc.tensor.matmul(
                                    scp[:, :n],
                                    kt[:, j * 128:(j + 1) * 128],
                                    qt[:, qc * 512 + 512 - n:(qc + 1) * 512],
                                    start=True, stop=True)
                                nc.scalar.activation(
                                    out=at[:, s, :n], in_=scp[:, :n],
                                    func=mybir.ActivationFunctionType.Exp,
                                    scale=ALPHA)
                        for s in range(2):
                            j = 2 * pg + s
                            if moff(j) or j == 4 * qc:
                                nc.vector.tensor_mul(
                                    at[:, s, :128], at[:, s, :128], tri)
                        ats[pg] = at

                    sd = None
                    for pg in range(min(la, npair)):
                        emit(pg)
                    for pg in range(npair):
                        if pg + la < npair:
                            emit(pg + la)
                        at = ats.pop(pg)
                        for s in range(2):
                            j = 2 * pg + s
                            off = moff(j)
                            nc.tensor.matmul(
                                ypsum[:, off:], vt[:, j, :],
                                at[:, s, :512 - off],
                                start=(j == 0), stop=(j == nkt - 1))
                        if pg < 2 * qc:          # full pair
                            if pg % 2 == 0:
                                sg = pbs.tile([128, 512], BF,
                                              name="sg", tag="sg")
                                nc.vector.tensor_add(
                                    sg, at[:, 0, :], at[:, 1, :])
                            else:                # merge into quad, then MM
                                nc.vector.tensor_add(sg, sg, at[:, 0, :])
                                nc.vector.tensor_add(sg, sg, at[:, 1, :])
                                nc.tensor.matmul(
                                    dpsum, ones_c, sg,
                                    start=(pg == 1), stop=False)
                        elif pg == 2 * qc:       # diagonal pair 0
                            sd = pbs.tile([128, 512], BF, name="sd", tag="sd")
                            nc.vector.tensor_scalar_add(sd, at[:, 0, :], 0.0)
                            nc.vector.tensor_add(
                                sd[:, 128:], sd[:, 128:], at[:, 1, :384])
                        else:                    # diagonal pair 1
                            nc.vector.tensor_add(
                                sd[:, 256:], sd[:, 256:], at[:, 0, :256])
                            nc.vector.tensor_add(
                                sd[:, 384:], sd[:, 384:], at[:, 1, :128])
                            nc.tensor.matmul(
                                dpsum, ones_c, sd, start=(qc == 0), stop=True)
                        yield
                        maybe_load(nxt)
                    rbs = pbr.tile([128, 512], BF, name="rbs", tag="rbs")
                    with nc.allow_low_precision("softmax recip bf16"):
                        nc.vector.reciprocal(out=rbs, in_=dpsum)
                    yf = pbr.tile([128, 512], BF, name="yf", tag="yf")
                    nc.vector.tensor_mul(yf, ypsum, rbs)
                    nc.vector.tensor_mul(yh[:, h, qs], ypsum, rbs)
                    nc.vector.tensor_sub(yl[:, h, qs], yf, yh[:, h, qs])

        # ---------------- schedule ----------------
        do_a = "A" in phases
        do_b = "B" in phases
        do_c = "C" in phases

        if do_a:
            for _ in a_chunks([0, 1, 2]):       # S1
                pass
            s1stk.close()
            psa_box.append(astk.enter_context(
                tc.tile_pool(name="psA", bufs=2, space="PSUM")))
        if do_a and do_b:
            with tc.tile_pool(name="psS2", bufs=3, space="PSUM") as pss2, \
                 tc.tile_pool(name="psY2", bufs=2, space="PSUM") as psy2, \
                 tc.tile_pool(name="psD2", bufs=1, space="PSUM") as psd2:
                _weave((a_chunks([3, 4, 5]), A_UNIT),
                       (b_heads([0, 1, 2, 3], (pss2, psy2, psd2, False),
                                tail_load=4), B_UNIT))   # S2
        elif do_a:
            for _ in a_chunks([3, 4, 5]):
                pass
        astk.close()                            # free QKV pools / PSUM

        pcw = bstk.enter_context(tc.tile_pool(name="pCw", bufs=1))
        wohp, wolp = [], []
        if do_c:
            for p in range(NH // 2):
                dd = slice(2 * p, 2 * p + 2)
                t1 = pcw.tile([128, 2, L], FP8, name=f"woh{p}", tag=f"woh{p}")
                nc.sync.dma_start(out=t1, in_=woh_d[:, dd, :])
                wohp.append(t1)
                t2 = pcw.tile([128, 2, L], FP8, name=f"wol{p}", tag=f"wol{p}")
                nc.sync.dma_start(out=t2, in_=wol_d[:, dd, :])
                wolp.append(t2)
        pss3 = bstk.enter_context(
            tc.tile_pool(name="psS3", bufs=2, space="PSUM"))
        psy3 = bstk.enter_context(
            tc.tile_pool(name="psY3", bufs=2, space="PSUM"))
        psd3 = bstk.enter_context(
            tc.tile_pool(name="psD3", bufs=2, space="PSUM"))
        if do_b:
            rest = [4, 5, 6, 7] if do_a else list(range(NH))
            for _ in b_heads(rest, (pss3, psy3, psd3, True)):   # S3
                pass

        # ---------------- S4: out-projection (reuses B pools) ----------------
        if do_c:
                for e in range(DT):
                    ec = slice(e * 128, (e + 1) * 128)
                    for qc in range(QC):
                        qs = slice(qc * 512, (qc + 1) * 512)
                        op = pss3.tile([128, 2, 512], F32, name="op",
                                       tag="scp")[:, 0, :]
                        for p in range(NH // 2):
                            dd = slice(2 * p, 2 * p + 2)
                            nc.tensor.matmul(
                                op, wohp[p][:, :, ec], yh[:, dd, qs],
                                start=(p == 0), stop=False, perf_mode=DR)
                            nc.tensor.matmul(
                                op, wohp[p][:, :, ec], yl[:, dd, qs],
                                start=False, stop=False, perf_mode=DR)
                            nc.tensor.matmul(
                                op, wolp[p][:, :, ec], yh[:, dd, qs],
                                start=False, stop=(p == NH // 2 - 1),
                                perf_mode=DR)
                        ot = pba.tile([128, 2, 512], BF, name="ot",
                                      tag="at")[:, 0, :]
                        nc.scalar.copy(out=ot, in_=op)
                        nc.sync.dma_start(
                            out=outT[e * 128:(e + 1) * 128, qs], in_=ot)
        bstk.close()
    nc.compile()
    return nc


_NC_CACHE = None


def _get_program():
    global _NC_CACHE
    if _NC_CACHE is None:
        _NC_CACHE = build_program()
    return _NC_CACHE


def _q8(a):
    return np.clip(a, -240.0, 240.0).astype(E4)


def _hilo(a):
    hi = _q8(a)
    lo = _q8(a - hi.astype(np.float32))
    return hi, lo


def _host_inputs(x, w_qkv, w_o):
    inv = 1.0 / (ROPE_BASE ** (np.arange(0, HD, 2, dtype=np.float64) / HD))
    ang = np.arange(L, dtype=np.float64)[:, None] * inv[None, :]
    chalf = np.tile(np.cos(ang), (1, 4)).astype(BF16)          # [L, 256]
    shalf = np.tile(np.sin(ang), (1, 4)).astype(BF16)
    p = np.arange(128)[:, None]
    f = np.arange(128)[None, :]
    tri = (p <= f).astype(BF16)

    def to_pdl(a, nt):  # [nt*128, cols] -> [128, nt, cols]
        return np.ascontiguousarray(
            a.reshape(nt, 128, a.shape[1]).transpose(1, 0, 2))

    xs = {}
    for b in range(B):
        xh, xl = _hilo(XS * x[b].T)
        xs[b] = (to_pdl(xh, DT), to_pdl(xl, DT))

    in_maps = []
    for c in range(8):
        b, g = c % 4, c // 4
        qr = w_qkv[g * DL:(g + 1) * DL]
        kr = w_qkv[D + g * DL:D + (g + 1) * DL]
        vr = w_qkv[2 * D + g * DL:2 * D + (g + 1) * DL]
        wqkvT = np.concatenate(
            [qr[:512], kr[:512], vr[:512],
             qr[512:], kr[512:], vr[512:]], axis=0).T  # [D, 3DL]
        wh, wl = _hilo(WS * wqkvT)
        woT = w_o[:, g * DL:(g + 1) * DL].T            # [DL, D]
        woh, wol = _hilo(OS * woT)
        in_maps.append({
            "xh": xs[b][0], "xl": xs[b][1],
            "wh": to_pdl(wh, DT), "wl": to_pdl(wl, DT),
            "woh": to_pdl(woh, NH), "wol": to_pdl(wol, NH),
            "chalf": chalf, "shalf": shalf, "tri": tri,
        })
    return in_maps


def kernel(x, w_qkv, w_o, _trace=False):
    x = np.asarray(x, dtype=np.float32)
    w_qkv = np.asarray(w_qkv, dtype=np.float32)
    w_o = np.asarray(w_o, dtype=np.float32)
    nc = _get_program()
    in_maps = _host_inputs(x, w_qkv, w_o)
    res = run_bass_kernel_spmd(nc, in_maps, core_ids=list(range(8)),
                               trace=_trace)
    kernel.last_result = res
    parts = [r["outT"].astype(np.float32) for r in res.results]
    inv_scale = np.float32(1.0 / OUT_SCALE)
    out = np.empty((B, L, D), dtype=np.float32)
    for b in range(B):
        out[b] = (parts[b] + parts[b + 4]).T * inv_scale
    return out
